# revision 1
# baseline (speedup 1.0000x reference)
"""Trainium2 Bass kernel for a LLaMA-style causal attention block.

Sharding (8 NeuronCores, one trn2 chip):
  - Tensor-parallel over heads: core c owns heads [4c, 4c+4) -> wq/wk/wv column
    slices [4096, 512]; computes qT/kT/v + RoPE + causal attention for its heads.
  - attnT [512, 2048] (bf16) is AllGather'd (chunked over 4 sq quarters, so comm
    overlaps compute) -> each core computes out[:, 512c:512c+512] = attn @ wo_cols.
  - Host concatenates the 8 column slices.

Layout trick: everything is computed transposed ([head_dim, seq]) so that no
on-device transposes are needed anywhere:
  qT/kT = w_h.T @ xT      (xT host-pretransposed)
  scoresT[sk, sq] = kT_tile.T @ qT     (softmax denom on DVE/GpSimd, not PE)
  attnT[hd, sq] = v_tile.T @ expT      (expT is exactly the scoresT layout)
  out[sq, cols] = attnT_full_tile.T @ wo_tile
RoPE is applied in the transposed layout with a DVE stream_shuffle partition
pair-swap. exp() needs no max-subtraction: scores are O(1) by construction.

Compute dtype bf16 (f32 PSUM accumulation), I/O f32.
"""

import math
import os
import sys

for _p in ("/opt/trn_rl_repo",):
    if os.path.isdir(_p) and _p not in sys.path:
        sys.path.insert(0, _p)

import numpy as np
import ml_dtypes

N_CORES = 8
B, S, D, H = 1, 2048, 4096, 32
HD = D // H          # 128
HPC = H // N_CORES   # 4 heads per core
CW = D // N_CORES    # 512 columns per core
NK = D // 128        # 32 contraction tiles
SQT = 512            # sq tile width
NSQ = S // SQT       # 4
SCALE = 1.0 / math.sqrt(HD)

_CACHE = {}
LAST_RESULT = None   # test harness reads exec_time_ns from here


def _build():
    import concourse.mybir as mybir
    import concourse.tile as tile
    from concourse import bacc, bass_isa

    dt = mybir.dt
    f32, bf16 = dt.float32, dt.bfloat16

    nc = bacc.Bacc("TRN2", target_bir_lowering=False, debug=False,
                   num_devices=N_CORES)

    xT = nc.dram_tensor("xT", [D, S], bf16, kind="ExternalInput").ap()
    wq = nc.dram_tensor("wq", [D, CW], bf16, kind="ExternalInput").ap()
    wk = nc.dram_tensor("wk", [D, CW], bf16, kind="ExternalInput").ap()
    wv = nc.dram_tensor("wv", [D, CW], bf16, kind="ExternalInput").ap()
    wo = nc.dram_tensor("wo", [D, CW], bf16, kind="ExternalInput").ap()
    cosT = nc.dram_tensor("cosT", [HD, S], bf16, kind="ExternalInput").ap()
    sinT = nc.dram_tensor("sinT", [HD, S], bf16, kind="ExternalInput").ap()
    ones = nc.dram_tensor("ones", [HD, 1], bf16, kind="ExternalInput").ap()
    masks = nc.dram_tensor("masks", [4, 128, SQT], bf16, kind="ExternalInput").ap()
    out = nc.dram_tensor("out", [S, CW], f32, kind="ExternalOutput").ap()

    swap_mask = []
    for i in range(16):
        swap_mask += [2 * i + 1, 2 * i]

    rg = [list(range(N_CORES))]

    with tile.TileContext(nc) as tc:
        with (
            tc.tile_pool(name="consts", bufs=1) as cpool,
            tc.tile_pool(name="xp", bufs=34) as xpool,
            tc.tile_pool(name="wqp", bufs=6) as wqp,
            tc.tile_pool(name="wkp", bufs=6) as wkp,
            tc.tile_pool(name="wvp", bufs=8) as wvp,
            tc.tile_pool(name="res", bufs=1) as res,
            tc.tile_pool(name="rope32", bufs=5) as rope32,
            tc.tile_pool(name="ropebf", bufs=6) as ropebf,
            tc.tile_pool(name="expp", bufs=8) as expp,
            tc.tile_pool(name="nrm", bufs=2) as nrm,
            tc.tile_pool(name="attnsb", bufs=4) as attnsb,
            tc.tile_pool(name="wop", bufs=1) as wop,
            tc.tile_pool(name="agsb", bufs=8) as agsb,
            tc.tile_pool(name="osb", bufs=5) as osb,
            tc.tile_pool(name="ps", bufs=8, space="PSUM") as ps,
            tc.tile_pool(name="dram", bufs=1, space="DRAM") as dram,
        ):
            # resident results of QKV+rope
            qrot = [res.tile([HD, S], bf16, name=f"qrot{h}") for h in range(HPC)]
            krot = [res.tile([HD, S], bf16, name=f"krot{h}") for h in range(HPC)]
            v_sb = [res.tile([128, CW], bf16, name=f"v{i}") for i in range(S // 128)]

            # AllGather bounce buffers (one per sq quarter)
            ag_in = [dram.tile([HPC * HD, SQT], bf16, name=f"agin{q}")
                     for q in range(NSQ)]
            ag_out = [dram.tile([D, SQT], bf16, addr_space="Shared",
                                name=f"agout{q}") for q in range(NSQ)]

            cos_sb = cpool.tile([HD, S], bf16, name="cos_sb")
            ones_sb = cpool.tile([HD, 1], bf16, name="ones_sb")
            sin_sb = cpool.tile([HD, S], bf16, name="sin_sb")
            mask_sb = [cpool.tile([128, SQT], bf16, name=f"mask{r}")
                       for r in range(4)]
            wo_sb = [wop.tile([128, CW], bf16, name=f"wo{d}") for d in range(NK)]

            def emit_qkv(st):
                sq0 = st * SQT
                q_ps = [ps.tile([128, SQT], f32, tag="b", name=f"qps{st}_{h}")
                        for h in range(HPC)]
                k_ps = [ps.tile([128, SQT], f32, tag="b", name=f"kps{st}_{h}")
                        for h in range(HPC)]
                x_tiles = []
                for d in range(NK):
                    xt = xpool.tile([128, SQT], bf16, tag="x", name=f"x{st}_{d}")
                    nc.sync.dma_start(xt[:], xT[d * 128:(d + 1) * 128,
                                                sq0:sq0 + SQT])
                    x_tiles.append(xt)
                    wqt = wqp.tile([128, CW], bf16, tag="wq", name=f"wq{st}_{d}")
                    nc.sync.dma_start(wqt[:], wq[d * 128:(d + 1) * 128, :])
                    wkt = wkp.tile([128, CW], bf16, tag="wk", name=f"wk{st}_{d}")
                    nc.sync.dma_start(wkt[:], wk[d * 128:(d + 1) * 128, :])
                    first, last = d == 0, d == NK - 1
                    for h in range(HPC):
                        nc.tensor.matmul(q_ps[h][:], wqt[:, h * HD:(h + 1) * HD],
                                         xt[:], start=first, stop=last)
                    for h in range(HPC):
                        nc.tensor.matmul(k_ps[h][:], wkt[:, h * HD:(h + 1) * HD],
                                         xt[:], start=first, stop=last)
                if st == 0:
                    # constants are first needed by RoPE / attention below;
                    # emitting them here keeps the first QKV DMAs in front
                    nc.sync.dma_start(cos_sb[:], cosT[:])
                    nc.sync.dma_start(sin_sb[:], sinT[:])
                    nc.sync.dma_start(ones_sb[:], ones[:])
                    for r in range(4):
                        nc.sync.dma_start(mask_sb[r][:], masks[r])
                # RoPE: rot = t*cos + shuffle(t)*sin'   (sin' sign-baked)
                for h in range(HPC):
                    for pst, rot in ((q_ps[h], qrot[h]), (k_ps[h], krot[h])):
                        tbf = ropebf.tile([128, SQT], bf16, tag="rbf",
                                          name=f"rbf{st}_{h}")
                        nc.scalar.copy(tbf[:], pst[:])
                        tsw = ropebf.tile([128, SQT], bf16, tag="rsw",
                                          name=f"rsw{st}_{h}")
                        nc.vector.stream_shuffle(tsw[:], tbf[:], swap_mask)
                        t1 = rope32.tile([128, SQT], f32, tag="r32",
                                         name=f"r1_{st}_{h}")
                        nc.vector.tensor_mul(t1[:], tbf[:],
                                             cos_sb[:, sq0:sq0 + SQT])
                        t2 = rope32.tile([128, SQT], f32, tag="r32",
                                         name=f"r2_{st}_{h}")
                        nc.vector.tensor_mul(t2[:], tsw[:],
                                             sin_sb[:, sq0:sq0 + SQT])
                        nc.vector.tensor_add(rot[:, sq0:sq0 + SQT], t1[:], t2[:])
                # V projection for this s range; all wv loads are issued
                # up front so the first V matmuls never wait on DMA
                wv_tiles = []
                for d in range(NK):
                    wvt = wvp.tile([128, CW], bf16, tag="wv", name=f"wv{st}_{d}")
                    nc.sync.dma_start(wvt[:], wv[d * 128:(d + 1) * 128, :])
                    wv_tiles.append(wvt)
                v_ps = [ps.tile([128, CW], f32, tag="b", name=f"vps{st}_{ss}")
                        for ss in range(4)]
                for d in range(NK):
                    first, last = d == 0, d == NK - 1
                    for ss in range(4):
                        nc.tensor.matmul(v_ps[ss][:],
                                         x_tiles[d][:, ss * 128:(ss + 1) * 128],
                                         wv_tiles[d][:], start=first, stop=last)
                for ss in range(4):
                    nc.scalar.copy(v_sb[st * 4 + ss][:], v_ps[ss][:])

            def emit_attention(sqT):
                sq0 = sqT * SQT
                nblk = 4 * (sqT + 1)
                a_tiles = []
                for h in range(HPC):
                    attn_ps = ps.tile([HD, SQT], f32, tag="b",
                                      name=f"aps{sqT}_{h}")
                    den_ps = ps.tile([1, SQT], f32, tag="b",
                                     name=f"dps{sqT}_{h}")
                    exp_tiles = []

                    def emit_pv(j, h=h, attn_ps=attn_ps, den_ps=den_ps,
                                exp_tiles=exp_tiles, nblk=nblk, sqT=sqT):
                        first, last = j == 0, j == nblk - 1
                        e, off = exp_tiles[j]
                        n = SQT - off
                        nc.tensor.matmul(attn_ps[:, off:SQT],
                                         v_sb[j][:, h * HD:(h + 1) * HD],
                                         e[:, 0:n],
                                         start=first, stop=last)
                        nc.tensor.matmul(den_ps[:, off:SQT], ones_sb[:],
                                         e[:, 0:n],
                                         start=first, stop=last)

                    for i in range(nblk):
                        r = i - 4 * sqT
                        # diagonal blocks: only sq >= sk is valid; skip the
                        # fully-masked leading columns entirely
                        off = max(0, r) * 128
                        n = SQT - off
                        sc = ps.tile([128, SQT], f32, tag="b",
                                     name=f"sc{sqT}_{h}_{i}")
                        nc.tensor.matmul(sc[:, 0:n],
                                         krot[h][:, i * 128:(i + 1) * 128],
                                         qrot[h][:, sq0 + off:sq0 + SQT],
                                         start=True, stop=True)
                        if r >= 0:  # triangular part within the first strip
                            nc.vector.tensor_add(sc[:, 0:n], sc[:, 0:n],
                                                 mask_sb[r][:, off:SQT])
                        e = expp.tile([128, SQT], bf16, tag="e",
                                      name=f"e{sqT}_{h}_{i}")
                        nc.scalar.activation(e[:, 0:n], sc[:, 0:n],
                                             mybir.ActivationFunctionType.Exp,
                                             scale=SCALE)
                        exp_tiles.append((e, off))
                        if i >= 2:
                            emit_pv(i - 2)
                    emit_pv(nblk - 2)
                    emit_pv(nblk - 1)

                    # evacuate PSUM right away so the next round's projections
                    # get their banks back without waiting on the normalize
                    rec = nrm.tile([1, SQT], f32, tag="rec",
                                   name=f"rec{sqT}_{h}")
                    nc.vector.reciprocal(rec[:], den_ps[:])
                    bc = nrm.tile([128, SQT], f32, tag="bc",
                                  name=f"bc{sqT}_{h}")
                    nc.gpsimd.partition_broadcast(bc[:], rec[:], channels=128)
                    a_sb = attnsb.tile([HD, SQT], bf16, tag="a",
                                       name=f"asb{sqT}_{h}")
                    nc.vector.tensor_mul(a_sb[:], attn_ps[:], bc[:])
                    a_tiles.append(a_sb)
                return a_tiles

            def emit_attn_stores(sqT, a_tiles):
                # stores are emitted one round late so they never sit at the
                # head of the in-order DMA queue blocking the next round's
                # ready-to-issue loads
                for h in range(HPC):
                    nc.sync.dma_start(ag_in[sqT][h * HD:(h + 1) * HD, :],
                                      a_tiles[h][:])
                nc.gpsimd.collective_compute(
                    "AllGather", mybir.AluOpType.bypass, replica_groups=rg,
                    ins=[ag_in[sqT].opt()], outs=[ag_out[sqT].opt()])

            # attention for round st is emitted after QKV round st+1 (its
            # matmuls fill PSUM-release stalls at QKV round boundaries); its
            # stores go out one round later still
            emit_qkv(0)
            pending = None
            for st in range(1, NSQ):
                emit_qkv(st)
                if pending is not None:
                    emit_attn_stores(st - 2, pending)
                pending = emit_attention(st - 1)
                if st == 1:
                    for d in range(NK):  # prefetch wo during attention
                        nc.sync.dma_start(wo_sb[d][:],
                                          wo[d * 128:(d + 1) * 128, :])
            emit_attn_stores(NSQ - 2, pending)
            pending = emit_attention(NSQ - 1)
            emit_attn_stores(NSQ - 1, pending)

            # ================= output projection =================
            pending_o = None
            for q in range(NSQ):
                o_ps = [ps.tile([128, CW], f32, tag="b", name=f"ops{q}_{ss}")
                        for ss in range(4)]
                for d in range(NK):
                    agt = agsb.tile([128, SQT], bf16, tag="ag",
                                    name=f"agt{q}_{d}")
                    nc.sync.dma_start(agt[:],
                                      ag_out[q][d * 128:(d + 1) * 128, :])
                    first, last = d == 0, d == NK - 1
                    for ss in range(4):
                        nc.tensor.matmul(o_ps[ss][:],
                                         agt[:, ss * 128:(ss + 1) * 128],
                                         wo_sb[d][:], start=first, stop=last)
                    if d == 4 and pending_o is not None:
                        # previous quarter's stores, emitted after this
                        # quarter's first loads (no DMA-queue blocking)
                        qq, tiles = pending_o
                        for ss in range(4):
                            nc.sync.dma_start(
                                out[qq * SQT + ss * 128:
                                    qq * SQT + (ss + 1) * 128, :],
                                tiles[ss][:])
                        pending_o = None
                o_tiles = []
                for ss in range(4):
                    o = osb.tile([128, CW], f32, tag="o", name=f"o{q}_{ss}")
                    nc.scalar.copy(o[:], o_ps[ss][:])
                    o_tiles.append(o)
                pending_o = (q, o_tiles)
            qq, tiles = pending_o
            for ss in range(4):
                nc.sync.dma_start(
                    out[qq * SQT + ss * 128:qq * SQT + (ss + 1) * 128, :],
                    tiles[ss][:])

    nc.compile()
    return nc


def _prep_inputs(x, wq, wk, wv, wo, freqs_cos, freqs_sin, mask):
    bf16 = ml_dtypes.bfloat16
    x2 = np.asarray(x, dtype=np.float32).reshape(S, D)
    xT = np.ascontiguousarray(x2.T).astype(bf16)
    cosT = np.repeat(np.asarray(freqs_cos, np.float32).T, 2, axis=0)
    sinT = np.repeat(np.asarray(freqs_sin, np.float32).T, 2, axis=0).copy()
    sinT[0::2] *= -1.0
    cosT = np.ascontiguousarray(cosT).astype(bf16)
    sinT = np.ascontiguousarray(sinT).astype(bf16)
    m2 = np.asarray(mask, np.float32).reshape(S, S)
    masks = np.stack([np.ascontiguousarray(m2[0:SQT, r * 128:(r + 1) * 128].T)
                      for r in range(4)]).astype(bf16)  # [4, 128, 512]
    in_maps = []
    for c in range(N_CORES):
        cols = slice(c * CW, (c + 1) * CW)
        in_maps.append({
            "xT": xT,
            "wq": np.ascontiguousarray(np.asarray(wq, np.float32)[:, cols]).astype(bf16),
            "wk": np.ascontiguousarray(np.asarray(wk, np.float32)[:, cols]).astype(bf16),
            "wv": np.ascontiguousarray(np.asarray(wv, np.float32)[:, cols]).astype(bf16),
            "wo": np.ascontiguousarray(np.asarray(wo, np.float32)[:, cols]).astype(bf16),
            "cosT": cosT,
            "ones": np.ones((HD, 1), bf16),
            "sinT": sinT,
            "masks": masks,
        })
    return in_maps


def kernel(x, wq, wk, wv, wo, freqs_cos, freqs_sin, mask):
    global LAST_RESULT
    from concourse.bass_utils import run_bass_kernel_spmd

    if "nc" not in _CACHE:
        _CACHE["nc"] = _build()
    nc = _CACHE["nc"]
    in_maps = _prep_inputs(x, wq, wk, wv, wo, freqs_cos, freqs_sin, mask)
    res = run_bass_kernel_spmd(nc, in_maps, core_ids=list(range(N_CORES)))
    LAST_RESULT = res
    out = np.concatenate([res.results[c]["out"] for c in range(N_CORES)],
                         axis=1)
    return out.reshape(B, S, D).astype(np.float32)



# revision 18
# speedup vs baseline: 1.0107x; 1.0107x over previous
"""Trainium2 Bass kernel for a LLaMA-style causal attention block.

Sharding (8 NeuronCores, one trn2 chip):
  - Tensor-parallel over heads: core c owns heads [4c, 4c+4) -> wq/wk/wv column
    slices [4096, 512]; computes qT/kT/v + RoPE + causal attention for its heads.
  - attnT [512, 2048] (bf16) is AllGather'd per sq quarter -> each core computes
    out[:, 512c:512c+512] = attn @ wo_cols.  Host concatenates the 8 slices.

Layout trick: everything is computed transposed ([head_dim, seq]) so that no
on-device transposes are needed anywhere:
  qT/kT = w_h.T @ xT      (xT host-pretransposed)
  scoresT[sk, sq] = kT_tile.T @ qT     (softmax denom via col-packed PE matmuls)
  attnT[hd, sq] = v_tile.T @ expT      (expT is exactly the scoresT layout)
  out[sq, cols] = attnT_full_tile.T @ wo_tile
RoPE is applied in the transposed layout with a DVE stream_shuffle partition
pair-swap. exp() needs no max-subtraction: scores are O(1) by construction.

Perf notes vs the first working version:
  - All HBM loads are chunked 4 d-tiles per DMA descriptor (fewer Sync-queue
    descriptors; the in-order queue stays ahead of the PE).
  - ag_out -> SBUF loads allocate from the same pool as the x chunks, so
    buffer-reuse (WAR) deps order them after the last QKV round's loads in the
    Sync queue; a pending AllGather can no longer head-of-line-block the
    loads that feed the PE.
  - attn stores + AllGather triggers ride the GpSimd queue, not Sync.
  - Softmax denominators use four col-group-packed N=128 matmuls
    (tile_position) that run concurrently in the PE array instead of one
    full-width M=1 matmul: ~4x less PE time for the denominator.
  - Reciprocal runs after the partition broadcast ([128,512], all DVE lanes)
    instead of before ([1,512], single lane).
  - QKV rounds >=1 run as separate q/k/v passes so PSUM evacuation (RoPE on
    ACT+DVE) of one pass hides under the next pass's matmuls.

Compute dtype bf16 (f32 PSUM accumulation), I/O f32.
"""

import math
import os
import sys

for _p in ("/opt/trn_rl_repo",):
    if os.path.isdir(_p) and _p not in sys.path:
        sys.path.insert(0, _p)

import numpy as np
import ml_dtypes

N_CORES = 8
B, S, D, H = 1, 2048, 4096, 32
HD = D // H          # 128
HPC = H // N_CORES   # 4 heads per core
CW = D // N_CORES    # 512 columns per core
NK = D // 128        # 32 contraction tiles
SQT = 512            # sq tile width
NSQ = S // SQT       # 4
CH = 4               # d-tiles per DMA chunk
NCH = NK // CH       # 8 chunks per round
SCALE = 1.0 / math.sqrt(HD)

_CACHE = {}
LAST_RESULT = None   # test harness reads exec_time_ns from here


def _build():
    import concourse.mybir as mybir
    import concourse.tile as tile
    from concourse import bacc

    dt = mybir.dt
    f32, bf16 = dt.float32, dt.bfloat16

    nc = bacc.Bacc("TRN2", target_bir_lowering=False, debug=False,
                   num_devices=N_CORES)

    xT = nc.dram_tensor("xT", [D, S], bf16, kind="ExternalInput").ap()
    wq = nc.dram_tensor("wq", [D, CW], bf16, kind="ExternalInput").ap()
    wk = nc.dram_tensor("wk", [D, CW], bf16, kind="ExternalInput").ap()
    wv = nc.dram_tensor("wv", [D, CW], bf16, kind="ExternalInput").ap()
    wo = nc.dram_tensor("wo", [D, CW], bf16, kind="ExternalInput").ap()
    cosT = nc.dram_tensor("cosT", [HD, S], bf16, kind="ExternalInput").ap()
    sinT = nc.dram_tensor("sinT", [HD, S], bf16, kind="ExternalInput").ap()
    ones = nc.dram_tensor("ones", [HD, 1], bf16, kind="ExternalInput").ap()
    onesf = nc.dram_tensor("onesf", [128, 128], f32, kind="ExternalInput").ap()
    dmask = nc.dram_tensor("dmask", [128, 1], f32, kind="ExternalInput").ap()
    masks = nc.dram_tensor("masks", [4, 128, SQT], bf16, kind="ExternalInput").ap()
    out = nc.dram_tensor("out", [S, CW], f32, kind="ExternalOutput").ap()

    swap_mask = []
    for i in range(16):
        swap_mask += [2 * i + 1, 2 * i]

    rg = [list(range(N_CORES))]

    def chunk_src(t, c, cols):
        # rows [512c, 512c+512) of a [D, ncols] dram tensor, laid out so that
        # d-tile j of the chunk lands at free columns [j*w, (j+1)*w)
        return t[512 * c:512 * (c + 1), cols].rearrange("(j p) s -> p j s", j=CH)

    def chunk_dst(tl, w):
        return tl[:].rearrange("p (j s) -> p j s", s=w)

    with tile.TileContext(nc) as tc:
        with (
            tc.tile_pool(name="consts", bufs=1) as cpool,
            tc.tile_pool(name="xp", bufs=9) as xpool,
            tc.tile_pool(name="wqp", bufs=3) as wqp,
            tc.tile_pool(name="wkp", bufs=3) as wkp,
            tc.tile_pool(name="wvp", bufs=2) as wvp,
            tc.tile_pool(name="res", bufs=1) as res,
            tc.tile_pool(name="rope32", bufs=4) as rope32,
            tc.tile_pool(name="ropebf", bufs=6) as ropebf,
            tc.tile_pool(name="expp", bufs=6) as expp,
            tc.tile_pool(name="nrm", bufs=4) as nrm,
            tc.tile_pool(name="attnsb", bufs=4) as attnsb,
            tc.tile_pool(name="wop", bufs=1) as wop,
            tc.tile_pool(name="osb", bufs=4) as osb,
            tc.tile_pool(name="ps", bufs=8, space="PSUM") as ps,
            tc.tile_pool(name="dram", bufs=1, space="DRAM") as dram,
        ):
            # resident results of QKV+rope
            qrot = [res.tile([HD, S], bf16, name=f"qrot{h}") for h in range(HPC)]
            krot = [res.tile([HD, S], bf16, name=f"krot{h}") for h in range(HPC)]
            v_sb = [res.tile([128, CW], bf16, name=f"v{i}") for i in range(S // 128)]

            # AllGather bounce buffers (one per sq quarter)
            ag_in = [dram.tile([HPC * HD, SQT], bf16, name=f"agin{q}")
                     for q in range(NSQ)]
            ag_out = [dram.tile([D, SQT], bf16, addr_space="Shared",
                                name=f"agout{q}") for q in range(NSQ)]

            cos_sb = cpool.tile([HD, S], bf16, name="cos_sb")
            ones_sb = cpool.tile([HD, 1], bf16, name="ones_sb")
            onesf_sb = cpool.tile([128, 128], f32, name="onesf_sb")
            dmask_sb = cpool.tile([128, 1], f32, name="dmask_sb")
            sin_sb = cpool.tile([HD, S], bf16, name="sin_sb")
            mask_sb = [cpool.tile([128, SQT], bf16, name=f"mask{r}")
                       for r in range(4)]
            wo_sb = [wop.tile([128, CH * CW], bf16, name=f"wo{c}")
                     for c in range(NCH)]

            def emit_rope(ps_tiles, rots, sq0):
                # rot = t*cos + shuffle(t)*sin'   (sin' sign-baked)
                for h in range(HPC):
                    tbf = ropebf.tile([128, SQT], bf16, tag="rbf",
                                      name=f"rbf{sq0}_{h}")
                    nc.scalar.copy(tbf[:], ps_tiles[h][:])
                    tsw = ropebf.tile([128, SQT], bf16, tag="rsw",
                                      name=f"rsw{sq0}_{h}")
                    nc.vector.stream_shuffle(tsw[:], tbf[:], swap_mask)
                    t1 = rope32.tile([128, SQT], f32, tag="r32",
                                     name=f"r1_{sq0}_{h}")
                    nc.vector.tensor_mul(t1[:], tbf[:],
                                         cos_sb[:, sq0:sq0 + SQT])
                    t2 = rope32.tile([128, SQT], f32, tag="r32",
                                     name=f"r2_{sq0}_{h}")
                    nc.vector.tensor_mul(t2[:], tsw[:],
                                         sin_sb[:, sq0:sq0 + SQT])
                    nc.vector.tensor_add(rots[h][:, sq0:sq0 + SQT], t1[:], t2[:])

            def emit_qkv(st):
                sq0 = st * SQT
                interleave = st == 0  # round 0 has no prefetch headroom:
                # q+k share each freshly landed chunk so the PE keeps pace
                # with the DMA issue rate
                x_tiles = []
                q_ps = [ps.tile([128, SQT], f32, tag="b", name=f"qps{st}_{h}")
                        for h in range(HPC)]
                if interleave:
                    k_ps = [ps.tile([128, SQT], f32, tag="b",
                                    name=f"kps{st}_{h}") for h in range(HPC)]
                for c in range(NCH):
                    xt = xpool.tile([128, CH * SQT], bf16, tag="x",
                                    name=f"x{st}_{c}")
                    nc.sync.dma_start(chunk_dst(xt, SQT),
                                      chunk_src(xT, c, slice(sq0, sq0 + SQT)))
                    x_tiles.append(xt)
                    wqt = wqp.tile([128, CH * CW], bf16, tag="wq",
                                   name=f"wq{st}_{c}")
                    nc.sync.dma_start(chunk_dst(wqt, CW),
                                      chunk_src(wq, c, slice(None)))
                    if interleave:
                        wkt = wkp.tile([128, CH * CW], bf16, tag="wk",
                                       name=f"wk{st}_{c}")
                        nc.sync.dma_start(chunk_dst(wkt, CW),
                                          chunk_src(wk, c, slice(None)))
                    for j in range(CH):
                        d = CH * c + j
                        first, last = d == 0, d == NK - 1
                        for h in range(HPC):
                            nc.tensor.matmul(
                                q_ps[h][:],
                                wqt[:, j * CW + h * HD:j * CW + (h + 1) * HD],
                                xt[:, j * SQT:(j + 1) * SQT],
                                start=first, stop=last)
                        if interleave:
                            for h in range(HPC):
                                nc.tensor.matmul(
                                    k_ps[h][:],
                                    wkt[:, j * CW + h * HD:j * CW + (h + 1) * HD],
                                    xt[:, j * SQT:(j + 1) * SQT],
                                    start=first, stop=last)
                if st == 0:
                    # constants are first needed by RoPE / attention below;
                    # emitting them here keeps the first QKV DMAs in front
                    nc.sync.dma_start(cos_sb[:], cosT[:])
                    nc.sync.dma_start(sin_sb[:], sinT[:])
                    nc.sync.dma_start(ones_sb[:], ones[:])
                    nc.sync.dma_start(onesf_sb[:], onesf[:])
                    nc.sync.dma_start(dmask_sb[:], dmask[:])
                    for r in range(4):
                        nc.sync.dma_start(mask_sb[r][:], masks[r])
                emit_rope(q_ps, qrot, sq0)
                if not interleave:
                    k_ps = [ps.tile([128, SQT], f32, tag="b",
                                    name=f"kps{st}_{h}") for h in range(HPC)]
                    for c in range(NCH):
                        wkt = wkp.tile([128, CH * CW], bf16, tag="wk",
                                       name=f"wk{st}_{c}")
                        nc.sync.dma_start(chunk_dst(wkt, CW),
                                          chunk_src(wk, c, slice(None)))
                        for j in range(CH):
                            d = CH * c + j
                            first, last = d == 0, d == NK - 1
                            for h in range(HPC):
                                nc.tensor.matmul(
                                    k_ps[h][:],
                                    wkt[:, j * CW + h * HD:j * CW + (h + 1) * HD],
                                    x_tiles[c][:, j * SQT:(j + 1) * SQT],
                                    start=first, stop=last)
                emit_rope(k_ps, krot, sq0)
                # V projection for this s range
                v_ps = [ps.tile([128, CW], f32, tag="b", name=f"vps{st}_{ss}")
                        for ss in range(4)]
                for c in range(NCH):
                    wvt = wvp.tile([128, CH * CW], bf16, tag="wv",
                                   name=f"wv{st}_{c}")
                    nc.sync.dma_start(chunk_dst(wvt, CW),
                                      chunk_src(wv, c, slice(None)))
                    for j in range(CH):
                        d = CH * c + j
                        first, last = d == 0, d == NK - 1
                        for ss in range(4):
                            nc.tensor.matmul(
                                v_ps[ss][:],
                                x_tiles[c][:, j * SQT + ss * 128:
                                           j * SQT + (ss + 1) * 128],
                                wvt[:, j * CW:(j + 1) * CW],
                                start=first, stop=last)
                for ss in range(4):
                    nc.scalar.copy(v_sb[st * 4 + ss][:], v_ps[ss][:])

            def emit_attention(sqT):
                sq0 = sqT * SQT
                nblk = 4 * (sqT + 1)
                pending_norm = [None]

                def emit_normalize(sqT, h, attn_ps, den_ps):
                    # evacuate denominator with the garbage (never-written)
                    # partitions zeroed via a per-partition scale, then one
                    # all-ones matmul broadcasts the per-column sums to all
                    # partitions; reciprocal runs on all 128 DVE lanes
                    den_sb = nrm.tile([128, SQT], f32, tag="nrm",
                                      name=f"den{sqT}_{h}")
                    nc.scalar.copy(den_sb[:], den_ps[:])
                    bc_ps = ps.tile([128, SQT], f32, tag="b",
                                    name=f"bcps{sqT}_{h}")
                    nc.tensor.matmul(bc_ps[:], onesf_sb[:], den_sb[:],
                                     start=True, stop=True)
                    rec = nrm.tile([128, SQT], f32, tag="nrm",
                                   name=f"rec{sqT}_{h}")
                    nc.vector.reciprocal(rec[:], bc_ps[:])
                    a_sb = attnsb.tile([HD, SQT], bf16, tag="a",
                                       name=f"asb{sqT}_{h}")
                    nc.vector.tensor_mul(a_sb[:], attn_ps[:], rec[:])
                    # store + collective ride the GpSimd queue so a pending
                    # AllGather can never block the Sync load queue
                    nc.gpsimd.dma_start(ag_in[sqT][h * HD:(h + 1) * HD, :],
                                        a_sb[:])

                for h in range(HPC):
                    attn_ps = ps.tile([HD, SQT], f32, tag="b",
                                      name=f"aps{sqT}_{h}")
                    den_ps = ps.tile([128, SQT], f32, tag="b",
                                     name=f"dps{sqT}_{h}")
                    # zero the whole bank so the elements the col-packed
                    # denominator matmuls never write stay 0 and the
                    # all-ones broadcast matmul sums only valid rows
                    nc.vector.memset(den_ps[:], 0.0)
                    exp_tiles = []

                    def emit_pv(j, h=h, attn_ps=attn_ps, den_ps=den_ps,
                                exp_tiles=exp_tiles, nblk=nblk, sqT=sqT):
                        e, off = exp_tiles[j]
                        n = SQT - off
                        nc.tensor.matmul(attn_ps[:, off:SQT],
                                         v_sb[j][:, h * HD:(h + 1) * HD],
                                         e[:, 0:n],
                                         start=j == 0, stop=j == nblk - 1)
                        # denominator: four col-group-packed M=1 matmuls run
                        # concurrently in the PE array (disjoint col groups)
                        for s in range(off // 128, 4):
                            nc.tensor.matmul(
                                den_ps[32 * s:32 * s + 1,
                                       128 * s:128 * (s + 1)],
                                ones_sb[:],
                                e[:, 128 * s - off:128 * (s + 1) - off],
                                start=j == 0, stop=j == 4 * sqT + s,
                                tile_position=(0, 32 * s))

                    for i in range(nblk):
                        r = i - 4 * sqT
                        # diagonal blocks: only sq >= sk is valid; skip the
                        # fully-masked leading columns entirely
                        off = max(0, r) * 128
                        n = SQT - off
                        sc = ps.tile([128, SQT], f32, tag="b",
                                     name=f"sc{sqT}_{h}_{i}")
                        nc.tensor.matmul(sc[:, 0:n],
                                         krot[h][:, i * 128:(i + 1) * 128],
                                         qrot[h][:, sq0 + off:sq0 + SQT],
                                         start=True, stop=True)
                        if r >= 0:  # triangular part within the first strip
                            nc.vector.tensor_add(sc[:, 0:n], sc[:, 0:n],
                                                 mask_sb[r][:, off:SQT])
                        e = expp.tile([128, SQT], bf16, tag="e",
                                      name=f"e{sqT}_{h}_{i}")
                        nc.scalar.activation(e[:, 0:n], sc[:, 0:n],
                                             mybir.ActivationFunctionType.Exp,
                                             scale=SCALE)
                        exp_tiles.append((e, off))
                        if i == 1 and pending_norm[0] is not None:
                            # previous head's normalize, deferred so its
                            # ACT-copy latency hides under this head's
                            # first score matmuls
                            emit_normalize(*pending_norm[0])
                            pending_norm[0] = None
                        if i >= 2:
                            emit_pv(i - 2)
                    emit_pv(nblk - 2)
                    emit_pv(nblk - 1)
                    pending_norm[0] = (sqT, h, attn_ps, den_ps)
                emit_normalize(*pending_norm[0])
                nc.gpsimd.collective_compute(
                    "AllGather", mybir.AluOpType.bypass, replica_groups=rg,
                    ins=[ag_in[sqT].opt()], outs=[ag_out[sqT].opt()])

            for st in range(NSQ):
                emit_qkv(st)
                if st == 1:
                    for c in range(NCH):  # prefetch wo during round 1
                        nc.sync.dma_start(chunk_dst(wo_sb[c], CW),
                                          chunk_src(wo, c, slice(None)))
                emit_attention(st)

            # ================= output projection =================
            # agt chunks allocate from the x pool: buffer-reuse deps place
            # their (AllGather-gated) DMAs after the last QKV round's loads
            # in the Sync queue
            pending_o = None
            for q in range(NSQ):
                o_ps = [ps.tile([128, CW], f32, tag="b", name=f"ops{q}_{ss}")
                        for ss in range(4)]
                for c in range(NCH):
                    agt = xpool.tile([128, CH * SQT], bf16, tag="x",
                                     name=f"agt{q}_{c}")
                    nc.sync.dma_start(chunk_dst(agt, SQT),
                                      chunk_src(ag_out[q], c, slice(None)))
                    for j in range(CH):
                        d = CH * c + j
                        first, last = d == 0, d == NK - 1
                        for ss in range(4):
                            nc.tensor.matmul(
                                o_ps[ss][:],
                                agt[:, j * SQT + ss * 128:
                                    j * SQT + (ss + 1) * 128],
                                wo_sb[c][:, j * CW:(j + 1) * CW],
                                start=first, stop=last)
                    if c == 1 and pending_o is not None:
                        # previous quarter's stores, emitted after this
                        # quarter's first loads (no DMA-queue blocking)
                        qq, tiles = pending_o
                        for ss in range(4):
                            nc.sync.dma_start(
                                out[qq * SQT + ss * 128:
                                    qq * SQT + (ss + 1) * 128, :],
                                tiles[ss][:])
                        pending_o = None
                o_tiles = []
                for ss in range(4):
                    o = osb.tile([128, CW], f32, tag="o", name=f"o{q}_{ss}")
                    nc.vector.tensor_copy(o[:], o_ps[ss][:])
                    o_tiles.append(o)
                pending_o = (q, o_tiles)
            qq, tiles = pending_o
            for ss in range(4):
                nc.sync.dma_start(
                    out[qq * SQT + ss * 128:qq * SQT + (ss + 1) * 128, :],
                    tiles[ss][:])

    nc.compile()
    return nc


def _prep_inputs(x, wq, wk, wv, wo, freqs_cos, freqs_sin, mask):
    bf16 = ml_dtypes.bfloat16
    x2 = np.asarray(x, dtype=np.float32).reshape(S, D)
    xT = np.ascontiguousarray(x2.T).astype(bf16)
    cosT = np.repeat(np.asarray(freqs_cos, np.float32).T, 2, axis=0)
    sinT = np.repeat(np.asarray(freqs_sin, np.float32).T, 2, axis=0).copy()
    sinT[0::2] *= -1.0
    cosT = np.ascontiguousarray(cosT).astype(bf16)
    sinT = np.ascontiguousarray(sinT).astype(bf16)
    m2 = np.asarray(mask, np.float32).reshape(S, S)
    masks = np.stack([np.ascontiguousarray(m2[0:SQT, r * 128:(r + 1) * 128].T)
                      for r in range(4)]).astype(bf16)  # [4, 128, 512]
    dmask = np.zeros((128, 1), np.float32)
    dmask[[0, 32, 64, 96], 0] = 1.0
    in_maps = []
    for c in range(N_CORES):
        cols = slice(c * CW, (c + 1) * CW)
        in_maps.append({
            "xT": xT,
            "wq": np.ascontiguousarray(np.asarray(wq, np.float32)[:, cols]).astype(bf16),
            "wk": np.ascontiguousarray(np.asarray(wk, np.float32)[:, cols]).astype(bf16),
            "wv": np.ascontiguousarray(np.asarray(wv, np.float32)[:, cols]).astype(bf16),
            "wo": np.ascontiguousarray(np.asarray(wo, np.float32)[:, cols]).astype(bf16),
            "cosT": cosT,
            "ones": np.ones((HD, 1), bf16),
            "onesf": np.ones((128, 128), np.float32),
            "dmask": dmask,
            "sinT": sinT,
            "masks": masks,
        })
    return in_maps


def kernel(x, wq, wk, wv, wo, freqs_cos, freqs_sin, mask):
    global LAST_RESULT
    from concourse.bass_utils import run_bass_kernel_spmd

    if "nc" not in _CACHE:
        _CACHE["nc"] = _build()
    nc = _CACHE["nc"]
    in_maps = _prep_inputs(x, wq, wk, wv, wo, freqs_cos, freqs_sin, mask)
    res = run_bass_kernel_spmd(nc, in_maps, core_ids=list(range(N_CORES)))
    LAST_RESULT = res
    out = np.concatenate([res.results[c]["out"] for c in range(N_CORES)],
                         axis=1)
    return out.reshape(B, S, D).astype(np.float32)


# revision 31
# speedup vs baseline: 1.0530x; 1.0419x over previous
"""Trainium2 Bass kernel for a LLaMA-style causal attention block.

Sharding (8 NeuronCores, one trn2 chip):
  - Tensor-parallel over heads: core c owns heads [4c, 4c+4) -> wq/wk/wv column
    slices [4096, 512]; computes qT/kT/v + RoPE + causal attention for its heads.
  - attnT [512, 2048] (bf16) is AllGather'd per sq quarter -> each core computes
    out[:, 512c:512c+512] = attn @ wo_cols.  Host concatenates the 8 slices.

Layout trick: everything is computed transposed ([head_dim, seq]) so that no
on-device transposes are needed anywhere:
  qT/kT = w_h.T @ xT      (xT host-pretransposed)
  scoresT[sk, sq] = kT_tile.T @ qT     (softmax denom via col-packed PE matmuls)
  attnT[hd, sq] = v_tile.T @ expT      (expT is exactly the scoresT layout)
  out[sq, cols] = attnT_full_tile.T @ wo_tile
RoPE is applied in the transposed layout with a DVE stream_shuffle partition
pair-swap. exp() needs no max-subtraction: scores are O(1) by construction.

Perf notes vs the first working version:
  - All HBM loads are chunked 4 d-tiles per DMA descriptor (fewer Sync-queue
    descriptors; the in-order queue stays ahead of the PE).
  - ag_out -> SBUF loads allocate from the same pool as the x chunks, so
    buffer-reuse (WAR) deps order them after the last QKV round's loads in the
    Sync queue; a pending AllGather can no longer head-of-line-block the
    loads that feed the PE.
  - attn stores + AllGather triggers ride the GpSimd queue, not Sync.
  - Softmax denominators use four col-group-packed N=128 matmuls
    (tile_position) that run concurrently in the PE array instead of one
    full-width M=1 matmul: ~4x less PE time for the denominator.
  - Reciprocal runs after the partition broadcast ([128,512], all DVE lanes)
    instead of before ([1,512], single lane).
  - QKV rounds >=1 run as separate q/k/v passes so PSUM evacuation (RoPE on
    ACT+DVE) of one pass hides under the next pass's matmuls.

Compute dtype bf16 (f32 PSUM accumulation), I/O f32.
"""

import math
import os
import sys

for _p in ("/opt/trn_rl_repo",):
    if os.path.isdir(_p) and _p not in sys.path:
        sys.path.insert(0, _p)

import numpy as np
import ml_dtypes

N_CORES = 8
B, S, D, H = 1, 2048, 4096, 32
HD = D // H          # 128
HPC = H // N_CORES   # 4 heads per core
CW = D // N_CORES    # 512 columns per core
NK = D // 128        # 32 contraction tiles
SQT = 512            # sq tile width
NSQ = S // SQT       # 4
CH = 4               # d-tiles per DMA chunk
NCH = NK // CH       # 8 chunks per round
SCALE = 1.0 / math.sqrt(HD)

_CACHE = {}
LAST_RESULT = None   # test harness reads exec_time_ns from here


def _build():
    import concourse.mybir as mybir
    import concourse.tile as tile
    from concourse import bacc

    dt = mybir.dt
    f32, bf16 = dt.float32, dt.bfloat16

    nc = bacc.Bacc("TRN2", target_bir_lowering=False, debug=False,
                   num_devices=N_CORES)

    xT = nc.dram_tensor("xT", [D, S], bf16, kind="ExternalInput").ap()
    wq = nc.dram_tensor("wq", [D, CW], bf16, kind="ExternalInput").ap()
    wk = nc.dram_tensor("wk", [D, CW], bf16, kind="ExternalInput").ap()
    wv = nc.dram_tensor("wv", [D, CW], bf16, kind="ExternalInput").ap()
    wo = nc.dram_tensor("wo", [D, CW], bf16, kind="ExternalInput").ap()
    cosT = nc.dram_tensor("cosT", [HD, S], bf16, kind="ExternalInput").ap()
    sinT = nc.dram_tensor("sinT", [HD, S], bf16, kind="ExternalInput").ap()
    ones = nc.dram_tensor("ones", [HD, 1], bf16, kind="ExternalInput").ap()
    onesb = nc.dram_tensor("onesb", [128, 128], bf16, kind="ExternalInput").ap()
    masks = nc.dram_tensor("masks", [4, 128, SQT], bf16, kind="ExternalInput").ap()
    out = nc.dram_tensor("out", [S, CW], f32, kind="ExternalOutput").ap()

    swap_mask = []
    for i in range(16):
        swap_mask += [2 * i + 1, 2 * i]

    rg = [list(range(N_CORES))]

    def chunk_src(t, c, cols):
        # rows [512c, 512c+512) of a [D, ncols] dram tensor, laid out so that
        # d-tile j of the chunk lands at free columns [j*w, (j+1)*w)
        return t[512 * c:512 * (c + 1), cols].rearrange("(j p) s -> p j s", j=CH)

    def chunk_dst(tl, w):
        return tl[:].rearrange("p (j s) -> p j s", s=w)

    with tile.TileContext(nc) as tc:
        with (
            tc.tile_pool(name="consts", bufs=1) as cpool,
            tc.tile_pool(name="xp", bufs=9) as xpool,
            tc.tile_pool(name="wqp", bufs=3) as wqp,
            tc.tile_pool(name="wkp", bufs=3) as wkp,
            tc.tile_pool(name="wvp", bufs=2) as wvp,
            tc.tile_pool(name="res", bufs=1) as res,
            tc.tile_pool(name="rope32", bufs=4) as rope32,
            tc.tile_pool(name="ropebf", bufs=6) as ropebf,
            tc.tile_pool(name="expp", bufs=6) as expp,
            tc.tile_pool(name="nrm", bufs=4) as nrm,
            tc.tile_pool(name="attnsb", bufs=4) as attnsb,
            tc.tile_pool(name="wop", bufs=1) as wop,
            tc.tile_pool(name="osb", bufs=4) as osb,
            tc.tile_pool(name="ps", bufs=8, space="PSUM") as ps,
            tc.tile_pool(name="dram", bufs=1, space="DRAM") as dram,
        ):
            # resident results of QKV+rope
            qrot = [res.tile([HD, S], bf16, name=f"qrot{h}") for h in range(HPC)]
            krot = [res.tile([HD, S], bf16, name=f"krot{h}") for h in range(HPC)]
            v_sb = [res.tile([128, CW], bf16, name=f"v{i}") for i in range(S // 128)]

            # AllGather bounce buffers (one per sq quarter)
            ag_in = [dram.tile([HPC * HD, SQT], bf16, name=f"agin{q}")
                     for q in range(NSQ)]
            ag_out = [dram.tile([D, SQT], bf16, addr_space="Shared",
                                name=f"agout{q}") for q in range(NSQ)]

            cos_sb = cpool.tile([HD, S], bf16, name="cos_sb")
            ones_sb = cpool.tile([HD, 1], bf16, name="ones_sb")
            onesb_sb = cpool.tile([128, 128], bf16, name="onesb_sb")
            sin_sb = cpool.tile([HD, S], bf16, name="sin_sb")
            mask_sb = [cpool.tile([128, SQT], bf16, name=f"mask{r}")
                       for r in range(4)]
            wo_sb = [wop.tile([128, CH * CW], bf16, name=f"wo{c}")
                     for c in range(NCH)]

            def emit_rope(ps_tiles, rots, sq0):
                # rot = t*cos + shuffle(t)*sin'   (sin' sign-baked)
                for h in range(HPC):
                    tbf = ropebf.tile([128, SQT], bf16, tag="rbf",
                                      name=f"rbf{sq0}_{h}")
                    nc.scalar.copy(tbf[:], ps_tiles[h][:])
                    tsw = ropebf.tile([128, SQT], bf16, tag="rsw",
                                      name=f"rsw{sq0}_{h}")
                    nc.vector.stream_shuffle(tsw[:], tbf[:], swap_mask)
                    t1 = rope32.tile([128, SQT], f32, tag="r32",
                                     name=f"r1_{sq0}_{h}")
                    nc.vector.tensor_mul(t1[:], tbf[:],
                                         cos_sb[:, sq0:sq0 + SQT])
                    t2 = rope32.tile([128, SQT], f32, tag="r32",
                                     name=f"r2_{sq0}_{h}")
                    nc.vector.tensor_mul(t2[:], tsw[:],
                                         sin_sb[:, sq0:sq0 + SQT])
                    nc.vector.tensor_add(rots[h][:, sq0:sq0 + SQT], t1[:], t2[:])

            def emit_qkv(st, tail=None):
                sq0 = st * SQT
                interleave = st == 0  # round 0 has no prefetch headroom:
                # q+k share each freshly landed chunk so the PE keeps pace
                # with the DMA issue rate
                tail = list(tail or [])
                x_tiles = []
                q_ps = [ps.tile([128, SQT], f32, tag="b", name=f"qps{st}_{h}")
                        for h in range(HPC)]
                if interleave:
                    k_ps = [ps.tile([128, SQT], f32, tag="b",
                                    name=f"kps{st}_{h}") for h in range(HPC)]
                for c in range(NCH):
                    xt = xpool.tile([128, CH * SQT], bf16, tag="x",
                                    name=f"x{st}_{c}")
                    x_tiles.append(xt)
                    wqt = wqp.tile([128, CH * CW], bf16, tag="wq",
                                   name=f"wq{st}_{c}")
                    if interleave:
                        wkt = wkp.tile([128, CH * CW], bf16, tag="wk",
                                       name=f"wk{st}_{c}")
                    if st == 0 and c == 0:
                        # cold start: per-d-tile loads so the first matmul
                        # waits on two small transfers, not three 512 KB ones
                        for j in range(CH):
                            dr = slice(128 * j, 128 * (j + 1))
                            nc.sync.dma_start(xt[:, j * SQT:(j + 1) * SQT],
                                              xT[dr, sq0:sq0 + SQT])
                            nc.sync.dma_start(wqt[:, j * CW:(j + 1) * CW],
                                              wq[dr, :])
                            nc.sync.dma_start(wkt[:, j * CW:(j + 1) * CW],
                                              wk[dr, :])
                    else:
                        nc.sync.dma_start(chunk_dst(xt, SQT),
                                          chunk_src(xT, c,
                                                    slice(sq0, sq0 + SQT)))
                        nc.sync.dma_start(chunk_dst(wqt, CW),
                                          chunk_src(wq, c, slice(None)))
                        if interleave:
                            nc.sync.dma_start(chunk_dst(wkt, CW),
                                              chunk_src(wk, c, slice(None)))
                    for j in range(CH):
                        d = CH * c + j
                        first, last = d == 0, d == NK - 1
                        for h in range(HPC):
                            nc.tensor.matmul(
                                q_ps[h][:],
                                wqt[:, j * CW + h * HD:j * CW + (h + 1) * HD],
                                xt[:, j * SQT:(j + 1) * SQT],
                                start=first, stop=last)
                        if interleave:
                            for h in range(HPC):
                                nc.tensor.matmul(
                                    k_ps[h][:],
                                    wkt[:, j * CW + h * HD:j * CW + (h + 1) * HD],
                                    xt[:, j * SQT:(j + 1) * SQT],
                                    start=first, stop=last)
                    if tail:
                        # previous attention round's deferred tail work
                        # (last PVs + normalize + AllGather) — emitted here
                        # so its exp/ACT dependencies have long completed
                        tail.pop(0)()
                if st == 0:
                    # constants are first needed by RoPE / attention below;
                    # emitting them here keeps the first QKV DMAs in front
                    nc.sync.dma_start(cos_sb[:], cosT[:])
                    nc.sync.dma_start(sin_sb[:], sinT[:])
                    nc.sync.dma_start(ones_sb[:], ones[:])
                    nc.sync.dma_start(onesb_sb[:], onesb[:])
                    for r in range(4):
                        nc.sync.dma_start(mask_sb[r][:], masks[r])
                emit_rope(q_ps, qrot, sq0)
                if not interleave:
                    k_ps = [ps.tile([128, SQT], f32, tag="b",
                                    name=f"kps{st}_{h}") for h in range(HPC)]
                    for c in range(NCH):
                        wkt = wkp.tile([128, CH * CW], bf16, tag="wk",
                                       name=f"wk{st}_{c}")
                        nc.sync.dma_start(chunk_dst(wkt, CW),
                                          chunk_src(wk, c, slice(None)))
                        for j in range(CH):
                            d = CH * c + j
                            first, last = d == 0, d == NK - 1
                            for h in range(HPC):
                                nc.tensor.matmul(
                                    k_ps[h][:],
                                    wkt[:, j * CW + h * HD:j * CW + (h + 1) * HD],
                                    x_tiles[c][:, j * SQT:(j + 1) * SQT],
                                    start=first, stop=last)
                emit_rope(k_ps, krot, sq0)
                # V projection for this s range
                v_ps = [ps.tile([128, CW], f32, tag="b", name=f"vps{st}_{ss}")
                        for ss in range(4)]
                for c in range(NCH):
                    wvt = wvp.tile([128, CH * CW], bf16, tag="wv",
                                   name=f"wv{st}_{c}")
                    nc.sync.dma_start(chunk_dst(wvt, CW),
                                      chunk_src(wv, c, slice(None)))
                    for j in range(CH):
                        d = CH * c + j
                        first, last = d == 0, d == NK - 1
                        for ss in range(4):
                            nc.tensor.matmul(
                                v_ps[ss][:],
                                x_tiles[c][:, j * SQT + ss * 128:
                                           j * SQT + (ss + 1) * 128],
                                wvt[:, j * CW:(j + 1) * CW],
                                start=first, stop=last)
                for ss in range(4):
                    nc.scalar.copy(v_sb[st * 4 + ss][:], v_ps[ss][:])

            def emit_attention(sqT):
                sq0 = sqT * SQT
                nblk = 4 * (sqT + 1)
                tail = []

                def emit_normalize(sqT, h, attn_ps, den_ps):
                    # evacuate denominator with the garbage (never-written)
                    # partitions zeroed via a per-partition scale, then one
                    # all-ones matmul broadcasts the per-column sums to all
                    # partitions; reciprocal runs on all 128 DVE lanes
                    den_sb = nrm.tile([128, SQT], bf16, tag="nrm",
                                      name=f"den{sqT}_{h}")
                    nc.scalar.copy(den_sb[:], den_ps[:])
                    bc_ps = ps.tile([128, SQT], f32, tag="b",
                                    name=f"bcps{sqT}_{h}")
                    nc.tensor.matmul(bc_ps[:], onesb_sb[:], den_sb[:],
                                     start=True, stop=True)
                    rec = nrm.tile([128, SQT], f32, tag="nrm",
                                   name=f"rec{sqT}_{h}")
                    nc.vector.reciprocal(rec[:], bc_ps[:])
                    a_sb = attnsb.tile([HD, SQT], bf16, tag="a",
                                       name=f"asb{sqT}_{h}")
                    nc.vector.tensor_mul(a_sb[:], attn_ps[:], rec[:])
                    # store + collective ride the GpSimd queue so a pending
                    # AllGather can never block the Sync load queue
                    nc.gpsimd.dma_start(ag_in[sqT][h * HD:(h + 1) * HD, :],
                                        a_sb[:])

                for h in range(HPC):
                    attn_ps = ps.tile([HD, SQT], f32, tag="b",
                                      name=f"aps{sqT}_{h}")
                    den_ps = ps.tile([128, SQT], f32, tag="b",
                                     name=f"dps{sqT}_{h}")
                    # zero the whole bank so the elements the col-packed
                    # denominator matmuls never write stay 0 and the
                    # all-ones broadcast matmul sums only valid rows
                    nc.vector.memset(den_ps[:], 0.0)
                    exp_tiles = []

                    def emit_pv(j, h=h, attn_ps=attn_ps, den_ps=den_ps,
                                exp_tiles=exp_tiles, nblk=nblk, sqT=sqT):
                        e, off = exp_tiles[j]
                        n = SQT - off
                        nc.tensor.matmul(attn_ps[:, off:SQT],
                                         v_sb[j][:, h * HD:(h + 1) * HD],
                                         e[:, 0:n],
                                         start=j == 0, stop=j == nblk - 1)
                        # denominator: four col-group-packed M=1 matmuls run
                        # concurrently in the PE array (disjoint col groups)
                        for s in range(off // 128, 4):
                            nc.tensor.matmul(
                                den_ps[32 * s:32 * s + 1,
                                       128 * s:128 * (s + 1)],
                                ones_sb[:],
                                e[:, 128 * s - off:128 * (s + 1) - off],
                                start=j == 0, stop=j == 4 * sqT + s,
                                tile_position=(0, 32 * s))

                    for i in range(nblk):
                        r = i - 4 * sqT
                        # diagonal blocks: only sq >= sk is valid; skip the
                        # fully-masked leading columns entirely
                        off = max(0, r) * 128
                        n = SQT - off
                        sc = ps.tile([128, SQT], f32, tag="b",
                                     name=f"sc{sqT}_{h}_{i}")
                        nc.tensor.matmul(sc[:, 0:n],
                                         krot[h][:, i * 128:(i + 1) * 128],
                                         qrot[h][:, sq0 + off:sq0 + SQT],
                                         start=True, stop=True)
                        if r >= 0:  # triangular part within the first strip
                            nc.vector.tensor_add(sc[:, 0:n], sc[:, 0:n],
                                                 mask_sb[r][:, off:SQT])
                        e = expp.tile([128, SQT], bf16, tag="e",
                                      name=f"e{sqT}_{h}_{i}")
                        nc.scalar.activation(e[:, 0:n], sc[:, 0:n],
                                             mybir.ActivationFunctionType.Exp,
                                             scale=SCALE)
                        exp_tiles.append((e, off))
                        if tail:
                            # previous head's deferred tail (last PVs +
                            # normalize): its exp/ACT deps completed while
                            # this head's first scores streamed
                            tail.pop(0)()
                        if i >= 2:
                            emit_pv(i - 2)
                    # defer this head's last two PVs + normalize past the
                    # next head's first score matmuls so they never stall
                    # the PE on the exp pipeline
                    is_last = h == HPC - 1

                    def _norm(sqT=sqT, h=h, attn_ps=attn_ps, den_ps=den_ps,
                              is_last=is_last):
                        emit_normalize(sqT, h, attn_ps, den_ps)
                        if is_last:
                            nc.gpsimd.collective_compute(
                                "AllGather", mybir.AluOpType.bypass,
                                replica_groups=rg,
                                ins=[ag_in[sqT].opt()],
                                outs=[ag_out[sqT].opt()])

                    tail = [lambda f=emit_pv, j=nblk - 2: f(j),
                            lambda f=emit_pv, j=nblk - 1: f(j),
                            _norm]
                return tail

            tail = None
            for st in range(NSQ):
                emit_qkv(st, tail)
                if st == 1:
                    for c in range(NCH):  # prefetch wo during round 1
                        nc.sync.dma_start(chunk_dst(wo_sb[c], CW),
                                          chunk_src(wo, c, slice(None)))
                tail = emit_attention(st)

            # ================= output projection =================
            # agt chunks allocate from the x pool: buffer-reuse deps place
            # their (AllGather-gated) DMAs after the last QKV round's loads
            # in the Sync queue
            pending_o = None
            tail = list(tail)
            for q in range(NSQ):
                o_ps = [ps.tile([128, CW], f32, tag="b", name=f"ops{q}_{ss}")
                        for ss in range(4)]
                for c in range(NCH):
                    agt = xpool.tile([128, CH * SQT], bf16, tag="x",
                                     name=f"agt{q}_{c}")
                    nc.sync.dma_start(chunk_dst(agt, SQT),
                                      chunk_src(ag_out[q], c, slice(None)))
                    for j in range(CH):
                        d = CH * c + j
                        first, last = d == 0, d == NK - 1
                        for ss in range(4):
                            nc.tensor.matmul(
                                o_ps[ss][:],
                                agt[:, j * SQT + ss * 128:
                                    j * SQT + (ss + 1) * 128],
                                wo_sb[c][:, j * CW:(j + 1) * CW],
                                start=first, stop=last)
                    if tail:
                        # last attention round's deferred tail (incl. its
                        # AllGather trigger)
                        tail.pop(0)()
                    if c == 1 and pending_o is not None:
                        # previous quarter's stores, emitted after this
                        # quarter's first loads (no DMA-queue blocking)
                        qq, tiles = pending_o
                        for ss in range(4):
                            nc.sync.dma_start(
                                out[qq * SQT + ss * 128:
                                    qq * SQT + (ss + 1) * 128, :],
                                tiles[ss][:])
                        pending_o = None
                o_tiles = []
                for ss in range(4):
                    o = osb.tile([128, CW], f32, tag="o", name=f"o{q}_{ss}")
                    nc.vector.tensor_copy(o[:], o_ps[ss][:])
                    o_tiles.append(o)
                pending_o = (q, o_tiles)
            qq, tiles = pending_o
            for ss in range(4):
                nc.sync.dma_start(
                    out[qq * SQT + ss * 128:qq * SQT + (ss + 1) * 128, :],
                    tiles[ss][:])

    nc.compile()
    return nc


def _prep_inputs(x, wq, wk, wv, wo, freqs_cos, freqs_sin, mask):
    bf16 = ml_dtypes.bfloat16
    x2 = np.asarray(x, dtype=np.float32).reshape(S, D)
    xT = np.ascontiguousarray(x2.T).astype(bf16)
    cosT = np.repeat(np.asarray(freqs_cos, np.float32).T, 2, axis=0)
    sinT = np.repeat(np.asarray(freqs_sin, np.float32).T, 2, axis=0).copy()
    sinT[0::2] *= -1.0
    cosT = np.ascontiguousarray(cosT).astype(bf16)
    sinT = np.ascontiguousarray(sinT).astype(bf16)
    m2 = np.asarray(mask, np.float32).reshape(S, S)
    masks = np.stack([np.ascontiguousarray(m2[0:SQT, r * 128:(r + 1) * 128].T)
                      for r in range(4)]).astype(bf16)  # [4, 128, 512]

    in_maps = []
    for c in range(N_CORES):
        cols = slice(c * CW, (c + 1) * CW)
        in_maps.append({
            "xT": xT,
            "wq": np.ascontiguousarray(np.asarray(wq, np.float32)[:, cols]).astype(bf16),
            "wk": np.ascontiguousarray(np.asarray(wk, np.float32)[:, cols]).astype(bf16),
            "wv": np.ascontiguousarray(np.asarray(wv, np.float32)[:, cols]).astype(bf16),
            "wo": np.ascontiguousarray(np.asarray(wo, np.float32)[:, cols]).astype(bf16),
            "cosT": cosT,
            "ones": np.ones((HD, 1), bf16),
            "onesb": np.ones((128, 128), bf16),
            "sinT": sinT,
            "masks": masks,
        })
    return in_maps


def kernel(x, wq, wk, wv, wo, freqs_cos, freqs_sin, mask):
    global LAST_RESULT
    from concourse.bass_utils import run_bass_kernel_spmd

    if "nc" not in _CACHE:
        _CACHE["nc"] = _build()
    nc = _CACHE["nc"]
    in_maps = _prep_inputs(x, wq, wk, wv, wo, freqs_cos, freqs_sin, mask)
    res = run_bass_kernel_spmd(nc, in_maps, core_ids=list(range(N_CORES)))
    LAST_RESULT = res
    out = np.concatenate([res.results[c]["out"] for c in range(N_CORES)],
                         axis=1)
    return out.reshape(B, S, D).astype(np.float32)


# revision 37
# speedup vs baseline: 1.0860x; 1.0313x over previous
"""Trainium2 Bass kernel for a LLaMA-style causal attention block.

Sharding (8 NeuronCores, one trn2 chip):
  - Tensor-parallel over heads: core c owns heads [4c, 4c+4) -> wq/wk/wv column
    slices [4096, 512]; computes qT/kT/v + RoPE + causal attention for its heads.
  - attnT [512, 2048] (bf16) is AllGather'd per sq quarter -> each core computes
    out[:, 512c:512c+512] = attn @ wo_cols.  Host concatenates the 8 slices.

Layout trick: everything is computed transposed ([head_dim, seq]) so that no
on-device transposes are needed anywhere:
  qT/kT = w_h.T @ xT      (xT host-pretransposed)
  scoresT[sk, sq] = kT_tile.T @ qT     (softmax denom via col-packed PE matmuls)
  attnT[hd, sq] = v_tile.T @ expT      (expT is exactly the scoresT layout)
  out[sq, cols] = attnT_full_tile.T @ wo_tile
RoPE is applied in the transposed layout with a DVE stream_shuffle partition
pair-swap. exp() needs no max-subtraction: scores are O(1) by construction.

Perf notes vs the first working version:
  - All HBM loads are chunked 4 d-tiles per DMA descriptor (fewer Sync-queue
    descriptors; the in-order queue stays ahead of the PE).
  - ag_out -> SBUF loads allocate from the same pool as the x chunks, so
    buffer-reuse (WAR) deps order them after the last QKV round's loads in the
    Sync queue; a pending AllGather can no longer head-of-line-block the
    loads that feed the PE.
  - attn stores + AllGather triggers ride the GpSimd queue, not Sync.
  - Softmax denominators use four col-group-packed N=128 matmuls
    (tile_position) that run concurrently in the PE array instead of one
    full-width M=1 matmul: ~4x less PE time for the denominator.
  - Reciprocal runs after the partition broadcast ([128,512], all DVE lanes)
    instead of before ([1,512], single lane).
  - QKV rounds >=1 run as separate q/k/v passes so PSUM evacuation (RoPE on
    ACT+DVE) of one pass hides under the next pass's matmuls.

Compute dtype bf16 (f32 PSUM accumulation), I/O f32.
"""

import math
import os
import sys

for _p in ("/opt/trn_rl_repo",):
    if os.path.isdir(_p) and _p not in sys.path:
        sys.path.insert(0, _p)

import numpy as np
import ml_dtypes

N_CORES = 8
B, S, D, H = 1, 2048, 4096, 32
HD = D // H          # 128
HPC = H // N_CORES   # 4 heads per core
CW = D // N_CORES    # 512 columns per core
NK = D // 128        # 32 contraction tiles
SQT = 512            # sq tile width
NSQ = S // SQT       # 4
CH = 4               # d-tiles per DMA chunk
NCH = NK // CH       # 8 chunks per round
SCALE = 1.0 / math.sqrt(HD)

_CACHE = {}
LAST_RESULT = None   # test harness reads exec_time_ns from here


def _build():
    import concourse.mybir as mybir
    import concourse.tile as tile
    from concourse import bacc

    dt = mybir.dt
    f32, bf16 = dt.float32, dt.bfloat16

    nc = bacc.Bacc("TRN2", target_bir_lowering=False, debug=False,
                   num_devices=N_CORES)

    xT = nc.dram_tensor("xT", [D, S], bf16, kind="ExternalInput").ap()
    wq = nc.dram_tensor("wq", [D, CW], bf16, kind="ExternalInput").ap()
    wk = nc.dram_tensor("wk", [D, CW], bf16, kind="ExternalInput").ap()
    wv = nc.dram_tensor("wv", [D, CW], bf16, kind="ExternalInput").ap()
    wo = nc.dram_tensor("wo", [D, CW], bf16, kind="ExternalInput").ap()
    cosT = nc.dram_tensor("cosT", [HD, S], bf16, kind="ExternalInput").ap()
    sinT = nc.dram_tensor("sinT", [HD, S], bf16, kind="ExternalInput").ap()
    ones = nc.dram_tensor("ones", [HD, 1], bf16, kind="ExternalInput").ap()
    onesb = nc.dram_tensor("onesb", [128, 128], bf16, kind="ExternalInput").ap()
    masks = nc.dram_tensor("masks", [4, 128, SQT], bf16, kind="ExternalInput").ap()
    out = nc.dram_tensor("out", [S, CW], f32, kind="ExternalOutput").ap()

    swap_mask = []
    for i in range(16):
        swap_mask += [2 * i + 1, 2 * i]

    rg = [list(range(N_CORES))]

    def chunk_src(t, c, cols):
        # rows [512c, 512c+512) of a [D, ncols] dram tensor, laid out so that
        # d-tile j of the chunk lands at free columns [j*w, (j+1)*w)
        return t[512 * c:512 * (c + 1), cols].rearrange("(j p) s -> p j s", j=CH)

    def chunk_dst(tl, w):
        return tl[:].rearrange("p (j s) -> p j s", s=w)

    with tile.TileContext(nc) as tc:
        with (
            tc.tile_pool(name="consts", bufs=1) as cpool,
            tc.tile_pool(name="xp", bufs=9) as xpool,
            tc.tile_pool(name="wqp", bufs=3) as wqp,
            tc.tile_pool(name="wkp", bufs=4) as wkp,
            tc.tile_pool(name="wvp", bufs=4) as wvp,
            tc.tile_pool(name="res", bufs=1) as res,
            tc.tile_pool(name="rope32", bufs=4) as rope32,
            tc.tile_pool(name="ropebf", bufs=6) as ropebf,
            tc.tile_pool(name="expp", bufs=6) as expp,
            tc.tile_pool(name="nrm", bufs=4) as nrm,
            tc.tile_pool(name="attnsb", bufs=4) as attnsb,
            tc.tile_pool(name="wop", bufs=4) as wop,
            tc.tile_pool(name="osb", bufs=4) as osb,
            tc.tile_pool(name="ps", bufs=8, space="PSUM") as ps,
            tc.tile_pool(name="dram", bufs=1, space="DRAM") as dram,
        ):
            # resident results of QKV+rope
            qrot = [res.tile([HD, S], bf16, name=f"qrot{h}") for h in range(HPC)]
            krot = [res.tile([HD, S], bf16, name=f"krot{h}") for h in range(HPC)]
            v_sb = [res.tile([128, CW], bf16, name=f"v{i}") for i in range(S // 128)]

            # AllGather bounce buffers (one per sq quarter)
            ag_in = [dram.tile([HPC * HD, SQT], bf16, name=f"agin{q}")
                     for q in range(NSQ)]
            ag_out = [dram.tile([D, SQT], bf16, addr_space="Shared",
                                name=f"agout{q}") for q in range(NSQ)]

            cos_sb = cpool.tile([HD, S], bf16, name="cos_sb")
            ones_sb = cpool.tile([HD, 1], bf16, name="ones_sb")
            onesb_sb = cpool.tile([128, 128], bf16, name="onesb_sb")
            sin_sb = cpool.tile([HD, S], bf16, name="sin_sb")
            mask_sb = [cpool.tile([128, SQT], bf16, name=f"mask{r}")
                       for r in range(4)]

            def emit_rope(ps_tiles, rots, sq0):
                # rot = t*cos + shuffle(t)*sin'   (sin' sign-baked)
                for h in range(HPC):
                    tbf = ropebf.tile([128, SQT], bf16, tag="rbf",
                                      name=f"rbf{sq0}_{h}")
                    nc.scalar.copy(tbf[:], ps_tiles[h][:])
                    tsw = ropebf.tile([128, SQT], bf16, tag="rsw",
                                      name=f"rsw{sq0}_{h}")
                    nc.vector.stream_shuffle(tsw[:], tbf[:], swap_mask)
                    t1 = rope32.tile([128, SQT], f32, tag="r32",
                                     name=f"r1_{sq0}_{h}")
                    nc.vector.tensor_mul(t1[:], tbf[:],
                                         cos_sb[:, sq0:sq0 + SQT])
                    t2 = rope32.tile([128, SQT], f32, tag="r32",
                                     name=f"r2_{sq0}_{h}")
                    nc.vector.tensor_mul(t2[:], tsw[:],
                                         sin_sb[:, sq0:sq0 + SQT])
                    nc.vector.tensor_add(rots[h][:, sq0:sq0 + SQT], t1[:], t2[:])

            def emit_qkv(st, tail=None):
                sq0 = st * SQT
                interleave = st == 0  # round 0 has no prefetch headroom:
                # q+k share each freshly landed chunk so the PE keeps pace
                # with the DMA issue rate
                tail = list(tail or [])
                x_tiles = []
                q_ps = [ps.tile([128, SQT], f32, tag="b", name=f"qps{st}_{h}")
                        for h in range(HPC)]
                if interleave:
                    k_ps = [ps.tile([128, SQT], f32, tag="b",
                                    name=f"kps{st}_{h}") for h in range(HPC)]
                for c in range(NCH):
                    xt = xpool.tile([128, CH * SQT], bf16, tag="x",
                                    name=f"x{st}_{c}")
                    x_tiles.append(xt)
                    wqt = wqp.tile([128, CH * CW], bf16, tag="wq",
                                   name=f"wq{st}_{c}")
                    if interleave:
                        wkt = wkp.tile([128, CH * CW], bf16, tag="wk",
                                       name=f"wk{st}_{c}")
                    if st == 0 and c == 0:
                        # cold start: per-d-tile loads so the first matmul
                        # waits on two small transfers, not three 512 KB ones
                        for j in range(CH):
                            dr = slice(128 * j, 128 * (j + 1))
                            nc.sync.dma_start(xt[:, j * SQT:(j + 1) * SQT],
                                              xT[dr, sq0:sq0 + SQT])
                            nc.sync.dma_start(wqt[:, j * CW:(j + 1) * CW],
                                              wq[dr, :])
                            nc.sync.dma_start(wkt[:, j * CW:(j + 1) * CW],
                                              wk[dr, :])
                    else:
                        nc.sync.dma_start(chunk_dst(xt, SQT),
                                          chunk_src(xT, c,
                                                    slice(sq0, sq0 + SQT)))
                        nc.sync.dma_start(chunk_dst(wqt, CW),
                                          chunk_src(wq, c, slice(None)))
                        if interleave:
                            nc.sync.dma_start(chunk_dst(wkt, CW),
                                              chunk_src(wk, c, slice(None)))
                    for j in range(CH):
                        d = CH * c + j
                        first, last = d == 0, d == NK - 1
                        for h in range(HPC):
                            nc.tensor.matmul(
                                q_ps[h][:],
                                wqt[:, j * CW + h * HD:j * CW + (h + 1) * HD],
                                xt[:, j * SQT:(j + 1) * SQT],
                                start=first, stop=last)
                        if interleave:
                            for h in range(HPC):
                                nc.tensor.matmul(
                                    k_ps[h][:],
                                    wkt[:, j * CW + h * HD:j * CW + (h + 1) * HD],
                                    xt[:, j * SQT:(j + 1) * SQT],
                                    start=first, stop=last)
                    if tail:
                        # previous attention round's deferred tail work
                        # (last PVs + normalize + AllGather) — emitted here
                        # so its exp/ACT dependencies have long completed
                        tail.pop(0)()
                if st == 0:
                    # constants are first needed by RoPE / attention below;
                    # emitting them here keeps the first QKV DMAs in front
                    nc.sync.dma_start(cos_sb[:], cosT[:])
                    nc.sync.dma_start(sin_sb[:], sinT[:])
                    nc.sync.dma_start(ones_sb[:], ones[:])
                    nc.sync.dma_start(onesb_sb[:], onesb[:])
                    for r in range(4):
                        nc.sync.dma_start(mask_sb[r][:], masks[r])
                emit_rope(q_ps, qrot, sq0)
                if not interleave:
                    k_ps = [ps.tile([128, SQT], f32, tag="b",
                                    name=f"kps{st}_{h}") for h in range(HPC)]
                    for c in range(NCH):
                        wkt = wkp.tile([128, CH * CW], bf16, tag="wk",
                                       name=f"wk{st}_{c}")
                        nc.sync.dma_start(chunk_dst(wkt, CW),
                                          chunk_src(wk, c, slice(None)))
                        for j in range(CH):
                            d = CH * c + j
                            first, last = d == 0, d == NK - 1
                            for h in range(HPC):
                                nc.tensor.matmul(
                                    k_ps[h][:],
                                    wkt[:, j * CW + h * HD:j * CW + (h + 1) * HD],
                                    x_tiles[c][:, j * SQT:(j + 1) * SQT],
                                    start=first, stop=last)
                emit_rope(k_ps, krot, sq0)
                # V projection for this s range
                v_ps = [ps.tile([128, CW], f32, tag="b", name=f"vps{st}_{ss}")
                        for ss in range(4)]
                for c in range(NCH):
                    wvt = wvp.tile([128, CH * CW], bf16, tag="wv",
                                   name=f"wv{st}_{c}")
                    nc.sync.dma_start(chunk_dst(wvt, CW),
                                      chunk_src(wv, c, slice(None)))
                    for j in range(CH):
                        d = CH * c + j
                        first, last = d == 0, d == NK - 1
                        for ss in range(4):
                            nc.tensor.matmul(
                                v_ps[ss][:],
                                x_tiles[c][:, j * SQT + ss * 128:
                                           j * SQT + (ss + 1) * 128],
                                wvt[:, j * CW:(j + 1) * CW],
                                start=first, stop=last)
                for ss in range(4):
                    nc.scalar.copy(v_sb[st * 4 + ss][:], v_ps[ss][:])

            def emit_attention(sqT):
                sq0 = sqT * SQT
                nblk = 4 * (sqT + 1)
                tail = []

                def emit_normalize(sqT, h, attn_ps, den_ps):
                    # evacuate denominator with the garbage (never-written)
                    # partitions zeroed via a per-partition scale, then one
                    # all-ones matmul broadcasts the per-column sums to all
                    # partitions; reciprocal runs on all 128 DVE lanes
                    den_sb = nrm.tile([128, SQT], bf16, tag="nrm",
                                      name=f"den{sqT}_{h}")
                    nc.scalar.copy(den_sb[:], den_ps[:])
                    bc_ps = ps.tile([128, SQT], f32, tag="b",
                                    name=f"bcps{sqT}_{h}")
                    nc.tensor.matmul(bc_ps[:], onesb_sb[:], den_sb[:],
                                     start=True, stop=True)
                    rec = nrm.tile([128, SQT], f32, tag="nrm",
                                   name=f"rec{sqT}_{h}")
                    # ~5x faster than reciprocal(); ~18 correct bits, far
                    # beyond the bf16 data feeding it. Keeps the DVE queue
                    # from head-of-line-blocking the attention mask adds.
                    nc.vector.reciprocal_approx_fast(rec[:], bc_ps[:])
                    a_sb = attnsb.tile([HD, SQT], bf16, tag="a",
                                       name=f"asb{sqT}_{h}")
                    nc.vector.tensor_mul(a_sb[:], attn_ps[:], rec[:])
                    # store + collective ride the GpSimd queue so a pending
                    # AllGather can never block the Sync load queue
                    nc.gpsimd.dma_start(ag_in[sqT][h * HD:(h + 1) * HD, :],
                                        a_sb[:])

                for h in range(HPC):
                    attn_ps = ps.tile([HD, SQT], f32, tag="b",
                                      name=f"aps{sqT}_{h}")
                    den_ps = ps.tile([128, SQT], f32, tag="b",
                                     name=f"dps{sqT}_{h}")
                    # zero the whole bank so the elements the col-packed
                    # denominator matmuls never write stay 0 and the
                    # all-ones broadcast matmul sums only valid rows
                    nc.vector.memset(den_ps[:], 0.0)
                    exp_tiles = []

                    def emit_pv(j, h=h, attn_ps=attn_ps, den_ps=den_ps,
                                exp_tiles=exp_tiles, nblk=nblk, sqT=sqT):
                        e, off = exp_tiles[j]
                        n = SQT - off
                        nc.tensor.matmul(attn_ps[:, off:SQT],
                                         v_sb[j][:, h * HD:(h + 1) * HD],
                                         e[:, 0:n],
                                         start=j == 0, stop=j == nblk - 1)
                        # denominator: four col-group-packed M=1 matmuls run
                        # concurrently in the PE array (disjoint col groups)
                        for s in range(off // 128, 4):
                            nc.tensor.matmul(
                                den_ps[32 * s:32 * s + 1,
                                       128 * s:128 * (s + 1)],
                                ones_sb[:],
                                e[:, 128 * s - off:128 * (s + 1) - off],
                                start=j == 0, stop=j == 4 * sqT + s,
                                tile_position=(0, 32 * s))

                    for i in range(nblk):
                        r = i - 4 * sqT
                        # diagonal blocks: only sq >= sk is valid; skip the
                        # fully-masked leading columns entirely
                        off = max(0, r) * 128
                        n = SQT - off
                        sc = ps.tile([128, SQT], f32, tag="b",
                                     name=f"sc{sqT}_{h}_{i}")
                        nc.tensor.matmul(sc[:, 0:n],
                                         krot[h][:, i * 128:(i + 1) * 128],
                                         qrot[h][:, sq0 + off:sq0 + SQT],
                                         start=True, stop=True)
                        if r >= 0:  # triangular part within the first strip
                            nc.vector.tensor_add(sc[:, 0:n], sc[:, 0:n],
                                                 mask_sb[r][:, off:SQT])
                        e = expp.tile([128, SQT], bf16, tag="e",
                                      name=f"e{sqT}_{h}_{i}")
                        nc.scalar.activation(e[:, 0:n], sc[:, 0:n],
                                             mybir.ActivationFunctionType.Exp,
                                             scale=SCALE)
                        exp_tiles.append((e, off))
                        if tail:
                            # previous head's deferred tail (last PVs +
                            # normalize): its exp/ACT deps completed while
                            # this head's first scores streamed
                            tail.pop(0)()
                        if i >= 2:
                            emit_pv(i - 2)
                    # defer this head's last two PVs + normalize past the
                    # next head's first score matmuls so they never stall
                    # the PE on the exp pipeline
                    is_last = h == HPC - 1

                    def _norm(sqT=sqT, h=h, attn_ps=attn_ps, den_ps=den_ps,
                              is_last=is_last):
                        emit_normalize(sqT, h, attn_ps, den_ps)
                        if is_last:
                            nc.gpsimd.collective_compute(
                                "AllGather", mybir.AluOpType.bypass,
                                replica_groups=rg,
                                ins=[ag_in[sqT].opt()],
                                outs=[ag_out[sqT].opt()])

                    tail = [lambda f=emit_pv, j=nblk - 2: f(j),
                            lambda f=emit_pv, j=nblk - 1: f(j),
                            _norm]
                return tail

            tail = None
            for st in range(NSQ):
                emit_qkv(st, tail)
                tail = emit_attention(st)

            # ================= output projection =================
            # agt chunks allocate from the x pool: buffer-reuse deps place
            # their (AllGather-gated) DMAs after the last QKV round's loads
            # in the Sync queue
            pending_o = None
            tail = list(tail)
            for q in range(NSQ):
                o_ps = [ps.tile([128, CW], f32, tag="b", name=f"ops{q}_{ss}")
                        for ss in range(4)]
                for c in range(NCH):
                    agt = xpool.tile([128, CH * SQT], bf16, tag="x",
                                     name=f"agt{q}_{c}")
                    nc.sync.dma_start(chunk_dst(agt, SQT),
                                      chunk_src(ag_out[q], c, slice(None)))
                    wot = wop.tile([128, CH * CW], bf16, tag="wo",
                                   name=f"wo{q}_{c}")
                    nc.sync.dma_start(chunk_dst(wot, CW),
                                      chunk_src(wo, c, slice(None)))
                    for j in range(CH):
                        d = CH * c + j
                        first, last = d == 0, d == NK - 1
                        for ss in range(4):
                            nc.tensor.matmul(
                                o_ps[ss][:],
                                agt[:, j * SQT + ss * 128:
                                    j * SQT + (ss + 1) * 128],
                                wot[:, j * CW:(j + 1) * CW],
                                start=first, stop=last)
                    if tail:
                        # last attention round's deferred tail (incl. its
                        # AllGather trigger)
                        tail.pop(0)()
                    if c == 1 and pending_o is not None:
                        # previous quarter's stores, emitted after this
                        # quarter's first loads (no DMA-queue blocking)
                        qq, tiles = pending_o
                        for ss in range(4):
                            nc.sync.dma_start(
                                out[qq * SQT + ss * 128:
                                    qq * SQT + (ss + 1) * 128, :],
                                tiles[ss][:])
                        pending_o = None
                o_tiles = []
                for ss in range(4):
                    o = osb.tile([128, CW], f32, tag="o", name=f"o{q}_{ss}")
                    nc.vector.tensor_copy(o[:], o_ps[ss][:])
                    o_tiles.append(o)
                pending_o = (q, o_tiles)
            qq, tiles = pending_o
            for ss in range(4):
                nc.sync.dma_start(
                    out[qq * SQT + ss * 128:qq * SQT + (ss + 1) * 128, :],
                    tiles[ss][:])

    nc.compile()
    return nc


def _prep_inputs(x, wq, wk, wv, wo, freqs_cos, freqs_sin, mask):
    bf16 = ml_dtypes.bfloat16
    x2 = np.asarray(x, dtype=np.float32).reshape(S, D)
    xT = np.ascontiguousarray(x2.T).astype(bf16)
    cosT = np.repeat(np.asarray(freqs_cos, np.float32).T, 2, axis=0)
    sinT = np.repeat(np.asarray(freqs_sin, np.float32).T, 2, axis=0).copy()
    sinT[0::2] *= -1.0
    cosT = np.ascontiguousarray(cosT).astype(bf16)
    sinT = np.ascontiguousarray(sinT).astype(bf16)
    m2 = np.asarray(mask, np.float32).reshape(S, S)
    masks = np.stack([np.ascontiguousarray(m2[0:SQT, r * 128:(r + 1) * 128].T)
                      for r in range(4)]).astype(bf16)  # [4, 128, 512]

    in_maps = []
    for c in range(N_CORES):
        cols = slice(c * CW, (c + 1) * CW)
        in_maps.append({
            "xT": xT,
            "wq": np.ascontiguousarray(np.asarray(wq, np.float32)[:, cols]).astype(bf16),
            "wk": np.ascontiguousarray(np.asarray(wk, np.float32)[:, cols]).astype(bf16),
            "wv": np.ascontiguousarray(np.asarray(wv, np.float32)[:, cols]).astype(bf16),
            "wo": np.ascontiguousarray(np.asarray(wo, np.float32)[:, cols]).astype(bf16),
            "cosT": cosT,
            "ones": np.ones((HD, 1), bf16),
            "onesb": np.ones((128, 128), bf16),
            "sinT": sinT,
            "masks": masks,
        })
    return in_maps


def kernel(x, wq, wk, wv, wo, freqs_cos, freqs_sin, mask):
    global LAST_RESULT
    from concourse.bass_utils import run_bass_kernel_spmd

    if "nc" not in _CACHE:
        _CACHE["nc"] = _build()
    nc = _CACHE["nc"]
    in_maps = _prep_inputs(x, wq, wk, wv, wo, freqs_cos, freqs_sin, mask)
    res = run_bass_kernel_spmd(nc, in_maps, core_ids=list(range(N_CORES)))
    LAST_RESULT = res
    out = np.concatenate([res.results[c]["out"] for c in range(N_CORES)],
                         axis=1)
    return out.reshape(B, S, D).astype(np.float32)


# revision 42
# speedup vs baseline: 1.0966x; 1.0098x over previous
"""Trainium2 Bass kernel for a LLaMA-style causal attention block.

Sharding (8 NeuronCores, one trn2 chip):
  - Tensor-parallel over heads: core c owns heads [4c, 4c+4) -> wq/wk/wv column
    slices [4096, 512]; computes qT/kT/v + RoPE + causal attention for its heads.
  - attnT [512, 2048] (bf16) is AllGather'd per sq quarter -> each core computes
    out[:, 512c:512c+512] = attn @ wo_cols.  Host concatenates the 8 slices.

Layout trick: everything is computed transposed ([head_dim, seq]) so that no
on-device transposes are needed anywhere:
  qT/kT = w_h.T @ xT      (xT host-pretransposed)
  scoresT[sk, sq] = kT_tile.T @ qT     (softmax denom via col-packed PE matmuls)
  attnT[hd, sq] = v_tile.T @ expT      (expT is exactly the scoresT layout)
  out[sq, cols] = attnT_full_tile.T @ wo_tile
RoPE is applied in the transposed layout with a DVE stream_shuffle partition
pair-swap. exp() needs no max-subtraction: scores are O(1) by construction.

Perf notes vs the first working version:
  - All HBM loads are chunked 4 d-tiles per DMA descriptor (fewer Sync-queue
    descriptors; the in-order queue stays ahead of the PE).
  - ag_out -> SBUF loads allocate from the same pool as the x chunks, so
    buffer-reuse (WAR) deps order them after the last QKV round's loads in the
    Sync queue; a pending AllGather can no longer head-of-line-block the
    loads that feed the PE.
  - attn stores + AllGather triggers ride the GpSimd queue, not Sync.
  - Softmax denominators use four col-group-packed N=128 matmuls
    (tile_position) that run concurrently in the PE array instead of one
    full-width M=1 matmul: ~4x less PE time for the denominator.
  - Reciprocal runs after the partition broadcast ([128,512], all DVE lanes)
    instead of before ([1,512], single lane).
  - QKV rounds >=1 run as separate q/k/v passes so PSUM evacuation (RoPE on
    ACT+DVE) of one pass hides under the next pass's matmuls.

Compute dtype bf16 (f32 PSUM accumulation), I/O f32.
"""

import math
import os
import sys

for _p in ("/opt/trn_rl_repo",):
    if os.path.isdir(_p) and _p not in sys.path:
        sys.path.insert(0, _p)

import numpy as np
import ml_dtypes

N_CORES = 8
B, S, D, H = 1, 2048, 4096, 32
HD = D // H          # 128
HPC = H // N_CORES   # 4 heads per core
CW = D // N_CORES    # 512 columns per core
NK = D // 128        # 32 contraction tiles
SQT = 512            # sq tile width
NSQ = S // SQT       # 4
CH = 4               # d-tiles per DMA chunk
NCH = NK // CH       # 8 chunks per round
SCALE = 1.0 / math.sqrt(HD)

_CACHE = {}
LAST_RESULT = None   # test harness reads exec_time_ns from here


def _build():
    import concourse.mybir as mybir
    import concourse.tile as tile
    from concourse import bacc

    dt = mybir.dt
    f32, bf16 = dt.float32, dt.bfloat16

    nc = bacc.Bacc("TRN2", target_bir_lowering=False, debug=False,
                   num_devices=N_CORES)

    xT = nc.dram_tensor("xT", [D, S], bf16, kind="ExternalInput").ap()
    wq = nc.dram_tensor("wq", [D, CW], bf16, kind="ExternalInput").ap()
    wk = nc.dram_tensor("wk", [D, CW], bf16, kind="ExternalInput").ap()
    wv = nc.dram_tensor("wv", [D, CW], bf16, kind="ExternalInput").ap()
    wo = nc.dram_tensor("wo", [D, CW], bf16, kind="ExternalInput").ap()
    cosT = nc.dram_tensor("cosT", [HD, S], bf16, kind="ExternalInput").ap()
    sinT = nc.dram_tensor("sinT", [HD, S], bf16, kind="ExternalInput").ap()
    ones = nc.dram_tensor("ones", [HD, 1], bf16, kind="ExternalInput").ap()
    onesb = nc.dram_tensor("onesb", [128, 128], bf16, kind="ExternalInput").ap()
    masks = nc.dram_tensor("masks", [4, 128, SQT], bf16, kind="ExternalInput").ap()
    out = nc.dram_tensor("out", [S, CW], f32, kind="ExternalOutput").ap()

    swap_mask = []
    for i in range(16):
        swap_mask += [2 * i + 1, 2 * i]

    rg = [list(range(N_CORES))]

    def chunk_src(t, c, cols):
        # rows [512c, 512c+512) of a [D, ncols] dram tensor, laid out so that
        # d-tile j of the chunk lands at free columns [j*w, (j+1)*w)
        return t[512 * c:512 * (c + 1), cols].rearrange("(j p) s -> p j s", j=CH)

    def chunk_dst(tl, w):
        return tl[:].rearrange("p (j s) -> p j s", s=w)

    with tile.TileContext(nc) as tc:
        with (
            tc.tile_pool(name="consts", bufs=1) as cpool,
            tc.tile_pool(name="xp", bufs=10) as xpool,
            tc.tile_pool(name="wqp", bufs=4) as wqp,
            tc.tile_pool(name="wkp", bufs=4) as wkp,
            tc.tile_pool(name="wvp", bufs=4) as wvp,
            tc.tile_pool(name="res", bufs=1) as res,
            tc.tile_pool(name="rope32", bufs=4) as rope32,
            tc.tile_pool(name="ropebf", bufs=4) as ropebf,
            tc.tile_pool(name="expp", bufs=5) as expp,
            tc.tile_pool(name="nrm", bufs=4) as nrm,
            tc.tile_pool(name="attnsb", bufs=4) as attnsb,
            tc.tile_pool(name="wop", bufs=4) as wop,
            tc.tile_pool(name="osb", bufs=4) as osb,
            tc.tile_pool(name="ps", bufs=8, space="PSUM") as ps,
            tc.tile_pool(name="dram", bufs=1, space="DRAM") as dram,
        ):
            # resident results of QKV+rope
            qrot = [res.tile([HD, S], bf16, name=f"qrot{h}") for h in range(HPC)]
            krot = [res.tile([HD, S], bf16, name=f"krot{h}") for h in range(HPC)]
            v_sb = [res.tile([128, CW], bf16, name=f"v{i}") for i in range(S // 128)]

            # AllGather bounce buffers (one per sq quarter)
            ag_in = [dram.tile([HPC * HD, SQT], bf16, name=f"agin{q}")
                     for q in range(NSQ)]
            ag_out = [dram.tile([D, SQT], bf16, addr_space="Shared",
                                name=f"agout{q}") for q in range(NSQ)]

            cos_sb = cpool.tile([HD, S], bf16, name="cos_sb")
            ones_sb = cpool.tile([HD, 1], bf16, name="ones_sb")
            onesb_sb = cpool.tile([128, 128], bf16, name="onesb_sb")
            sin_sb = cpool.tile([HD, S], bf16, name="sin_sb")
            mask_sb = [cpool.tile([128, SQT], bf16, name=f"mask{r}")
                       for r in range(4)]

            def emit_rope(ps_tiles, rots, sq0):
                # rot = t*cos + shuffle(t)*sin'   (sin' sign-baked)
                for h in range(HPC):
                    tbf = ropebf.tile([128, SQT], bf16, tag="rbf",
                                      name=f"rbf{sq0}_{h}")
                    nc.scalar.copy(tbf[:], ps_tiles[h][:])
                    tsw = ropebf.tile([128, SQT], bf16, tag="rsw",
                                      name=f"rsw{sq0}_{h}")
                    nc.vector.stream_shuffle(tsw[:], tbf[:], swap_mask)
                    t1 = rope32.tile([128, SQT], f32, tag="r32",
                                     name=f"r1_{sq0}_{h}")
                    nc.vector.tensor_mul(t1[:], tbf[:],
                                         cos_sb[:, sq0:sq0 + SQT])
                    t2 = rope32.tile([128, SQT], f32, tag="r32",
                                     name=f"r2_{sq0}_{h}")
                    nc.vector.tensor_mul(t2[:], tsw[:],
                                         sin_sb[:, sq0:sq0 + SQT])
                    nc.vector.tensor_add(rots[h][:, sq0:sq0 + SQT], t1[:], t2[:])

            def emit_qkv(st, tail=None):
                sq0 = st * SQT
                interleave = st == 0  # round 0 has no prefetch headroom:
                # q+k share each freshly landed chunk so the PE keeps pace
                # with the DMA issue rate
                tail = list(tail or [])
                x_tiles = []
                q_ps = [ps.tile([128, SQT], f32, tag="b", name=f"qps{st}_{h}")
                        for h in range(HPC)]
                if interleave:
                    k_ps = [ps.tile([128, SQT], f32, tag="b",
                                    name=f"kps{st}_{h}") for h in range(HPC)]
                for c in range(NCH):
                    xt = xpool.tile([128, CH * SQT], bf16, tag="x",
                                    name=f"x{st}_{c}")
                    x_tiles.append(xt)
                    wqt = wqp.tile([128, CH * CW], bf16, tag="wq",
                                   name=f"wq{st}_{c}")
                    if interleave:
                        wkt = wkp.tile([128, CH * CW], bf16, tag="wk",
                                       name=f"wk{st}_{c}")
                    if st == 0 and c <= 1:
                        # cold start: per-d-tile loads so the first matmul
                        # waits on two small transfers, not three 512 KB ones
                        for j in range(CH):
                            dr = slice(512 * c + 128 * j,
                                       512 * c + 128 * (j + 1))
                            nc.sync.dma_start(xt[:, j * SQT:(j + 1) * SQT],
                                              xT[dr, sq0:sq0 + SQT])
                            nc.sync.dma_start(wqt[:, j * CW:(j + 1) * CW],
                                              wq[dr, :])
                            nc.sync.dma_start(wkt[:, j * CW:(j + 1) * CW],
                                              wk[dr, :])
                    else:
                        nc.sync.dma_start(chunk_dst(xt, SQT),
                                          chunk_src(xT, c,
                                                    slice(sq0, sq0 + SQT)))
                        nc.sync.dma_start(chunk_dst(wqt, CW),
                                          chunk_src(wq, c, slice(None)))
                        if interleave:
                            nc.sync.dma_start(chunk_dst(wkt, CW),
                                              chunk_src(wk, c, slice(None)))
                    for j in range(CH):
                        d = CH * c + j
                        first, last = d == 0, d == NK - 1
                        for h in range(HPC):
                            nc.tensor.matmul(
                                q_ps[h][:],
                                wqt[:, j * CW + h * HD:j * CW + (h + 1) * HD],
                                xt[:, j * SQT:(j + 1) * SQT],
                                start=first, stop=last)
                        if interleave:
                            for h in range(HPC):
                                nc.tensor.matmul(
                                    k_ps[h][:],
                                    wkt[:, j * CW + h * HD:j * CW + (h + 1) * HD],
                                    xt[:, j * SQT:(j + 1) * SQT],
                                    start=first, stop=last)
                    if tail:
                        # previous attention round's deferred tail work
                        # (last PVs + normalize + AllGather) — emitted here
                        # so its exp/ACT dependencies have long completed
                        tail.pop(0)()
                if st == 0:
                    # constants are first needed by RoPE / attention below;
                    # emitting them here keeps the first QKV DMAs in front
                    nc.sync.dma_start(cos_sb[:], cosT[:])
                    nc.sync.dma_start(sin_sb[:], sinT[:])
                    nc.sync.dma_start(ones_sb[:], ones[:])
                    nc.sync.dma_start(onesb_sb[:], onesb[:])
                    for r in range(4):
                        nc.sync.dma_start(mask_sb[r][:], masks[r])
                emit_rope(q_ps, qrot, sq0)
                if not interleave:
                    k_ps = [ps.tile([128, SQT], f32, tag="b",
                                    name=f"kps{st}_{h}") for h in range(HPC)]
                    for c in range(NCH):
                        wkt = wkp.tile([128, CH * CW], bf16, tag="wk",
                                       name=f"wk{st}_{c}")
                        nc.sync.dma_start(chunk_dst(wkt, CW),
                                          chunk_src(wk, c, slice(None)))
                        for j in range(CH):
                            d = CH * c + j
                            first, last = d == 0, d == NK - 1
                            for h in range(HPC):
                                nc.tensor.matmul(
                                    k_ps[h][:],
                                    wkt[:, j * CW + h * HD:j * CW + (h + 1) * HD],
                                    x_tiles[c][:, j * SQT:(j + 1) * SQT],
                                    start=first, stop=last)
                emit_rope(k_ps, krot, sq0)
                # V projection for this s range
                v_ps = [ps.tile([128, CW], f32, tag="b", name=f"vps{st}_{ss}")
                        for ss in range(4)]
                for c in range(NCH):
                    wvt = wvp.tile([128, CH * CW], bf16, tag="wv",
                                   name=f"wv{st}_{c}")
                    nc.sync.dma_start(chunk_dst(wvt, CW),
                                      chunk_src(wv, c, slice(None)))
                    for j in range(CH):
                        d = CH * c + j
                        first, last = d == 0, d == NK - 1
                        for ss in range(4):
                            nc.tensor.matmul(
                                v_ps[ss][:],
                                x_tiles[c][:, j * SQT + ss * 128:
                                           j * SQT + (ss + 1) * 128],
                                wvt[:, j * CW:(j + 1) * CW],
                                start=first, stop=last)
                for ss in range(4):
                    nc.scalar.copy(v_sb[st * 4 + ss][:], v_ps[ss][:])

            def emit_attention(sqT):
                sq0 = sqT * SQT
                nblk = 4 * (sqT + 1)
                tail = []

                def emit_normalize(sqT, h, attn_ps, den_ps):
                    # evacuate denominator with the garbage (never-written)
                    # partitions zeroed via a per-partition scale, then one
                    # all-ones matmul broadcasts the per-column sums to all
                    # partitions; reciprocal runs on all 128 DVE lanes
                    den_sb = nrm.tile([128, SQT], bf16, tag="nrm",
                                      name=f"den{sqT}_{h}")
                    nc.scalar.copy(den_sb[:], den_ps[:])
                    bc_ps = ps.tile([128, SQT], f32, tag="b",
                                    name=f"bcps{sqT}_{h}")
                    nc.tensor.matmul(bc_ps[:], onesb_sb[:], den_sb[:],
                                     start=True, stop=True)
                    rec = nrm.tile([128, SQT], f32, tag="nrm",
                                   name=f"rec{sqT}_{h}")
                    # ~5x faster than reciprocal(); ~18 correct bits, far
                    # beyond the bf16 data feeding it. Keeps the DVE queue
                    # from head-of-line-blocking the attention mask adds.
                    nc.vector.reciprocal_approx_fast(rec[:], bc_ps[:])
                    a_sb = attnsb.tile([HD, SQT], bf16, tag="a",
                                       name=f"asb{sqT}_{h}")
                    nc.vector.tensor_mul(a_sb[:], attn_ps[:], rec[:])
                    # store + collective ride the GpSimd queue so a pending
                    # AllGather can never block the Sync load queue
                    nc.gpsimd.dma_start(ag_in[sqT][h * HD:(h + 1) * HD, :],
                                        a_sb[:])

                for h in range(HPC):
                    attn_ps = ps.tile([HD, SQT], f32, tag="b",
                                      name=f"aps{sqT}_{h}")
                    den_ps = ps.tile([128, SQT], f32, tag="b",
                                     name=f"dps{sqT}_{h}")
                    # zero the whole bank so the elements the col-packed
                    # denominator matmuls never write stay 0 and the
                    # all-ones broadcast matmul sums only valid rows
                    nc.vector.memset(den_ps[:], 0.0)
                    exp_tiles = []

                    def emit_pv(j, h=h, attn_ps=attn_ps, den_ps=den_ps,
                                exp_tiles=exp_tiles, nblk=nblk, sqT=sqT):
                        e, off = exp_tiles[j]
                        n = SQT - off
                        nc.tensor.matmul(attn_ps[:, off:SQT],
                                         v_sb[j][:, h * HD:(h + 1) * HD],
                                         e[:, 0:n],
                                         start=j == 0, stop=j == nblk - 1)
                        # denominator: four col-group-packed M=1 matmuls run
                        # concurrently in the PE array (disjoint col groups)
                        for s in range(off // 128, 4):
                            nc.tensor.matmul(
                                den_ps[32 * s:32 * s + 1,
                                       128 * s:128 * (s + 1)],
                                ones_sb[:],
                                e[:, 128 * s - off:128 * (s + 1) - off],
                                start=j == 0, stop=j == 4 * sqT + s,
                                tile_position=(0, 32 * s))

                    for i in range(nblk):
                        r = i - 4 * sqT
                        # diagonal blocks: only sq >= sk is valid; skip the
                        # fully-masked leading columns entirely
                        off = max(0, r) * 128
                        n = SQT - off
                        sc = ps.tile([128, SQT], f32, tag="b",
                                     name=f"sc{sqT}_{h}_{i}")
                        nc.tensor.matmul(sc[:, 0:n],
                                         krot[h][:, i * 128:(i + 1) * 128],
                                         qrot[h][:, sq0 + off:sq0 + SQT],
                                         start=True, stop=True)
                        if r >= 0:  # triangular part within the first strip
                            nc.vector.tensor_add(sc[:, 0:n], sc[:, 0:n],
                                                 mask_sb[r][:, off:SQT])
                        e = expp.tile([128, SQT], bf16, tag="e",
                                      name=f"e{sqT}_{h}_{i}")
                        nc.scalar.activation(e[:, 0:n], sc[:, 0:n],
                                             mybir.ActivationFunctionType.Exp,
                                             scale=SCALE)
                        exp_tiles.append((e, off))
                        if tail:
                            # previous head's deferred tail (last PVs +
                            # normalize): its exp/ACT deps completed while
                            # this head's first scores streamed
                            tail.pop(0)()
                        if i >= 2:
                            emit_pv(i - 2)
                    # defer this head's last two PVs + normalize past the
                    # next head's first score matmuls so they never stall
                    # the PE on the exp pipeline
                    is_last = h == HPC - 1

                    def _norm(sqT=sqT, h=h, attn_ps=attn_ps, den_ps=den_ps,
                              is_last=is_last):
                        emit_normalize(sqT, h, attn_ps, den_ps)
                        if is_last:
                            nc.gpsimd.collective_compute(
                                "AllGather", mybir.AluOpType.bypass,
                                replica_groups=rg,
                                ins=[ag_in[sqT].opt()],
                                outs=[ag_out[sqT].opt()])

                    tail = [lambda f=emit_pv, j=nblk - 2: f(j),
                            lambda f=emit_pv, j=nblk - 1: f(j),
                            _norm]
                return tail

            tail = None
            for st in range(NSQ):
                emit_qkv(st, tail)
                tail = emit_attention(st)

            # ================= output projection =================
            # agt chunks allocate from the x pool: buffer-reuse deps place
            # their (AllGather-gated) DMAs after the last QKV round's loads
            # in the Sync queue
            pending_o = None
            # flush the last attention round's deferred tail before the
            # o_ps allocations: the normalize must release its PSUM banks
            # or the allocator blocks the first out-proj matmuls on it
            for f in list(tail):
                f()
            tail = []
            for q in range(NSQ):
                o_ps = [ps.tile([128, CW], f32, tag="b", name=f"ops{q}_{ss}")
                        for ss in range(4)]
                for c in range(NCH):
                    agt = xpool.tile([128, CH * SQT], bf16, tag="x",
                                     name=f"agt{q}_{c}")
                    nc.sync.dma_start(chunk_dst(agt, SQT),
                                      chunk_src(ag_out[q], c, slice(None)))
                    wot = wop.tile([128, CH * CW], bf16, tag="wo",
                                   name=f"wo{q}_{c}")
                    nc.sync.dma_start(chunk_dst(wot, CW),
                                      chunk_src(wo, c, slice(None)))
                    for j in range(CH):
                        d = CH * c + j
                        first, last = d == 0, d == NK - 1
                        for ss in range(4):
                            nc.tensor.matmul(
                                o_ps[ss][:],
                                agt[:, j * SQT + ss * 128:
                                    j * SQT + (ss + 1) * 128],
                                wot[:, j * CW:(j + 1) * CW],
                                start=first, stop=last)
                    if tail:
                        # last attention round's deferred tail (incl. its
                        # AllGather trigger)
                        tail.pop(0)()
                    if c == 1 and pending_o is not None:
                        # previous quarter's stores, emitted after this
                        # quarter's first loads (no DMA-queue blocking)
                        qq, tiles = pending_o
                        for ss in range(4):
                            nc.sync.dma_start(
                                out[qq * SQT + ss * 128:
                                    qq * SQT + (ss + 1) * 128, :],
                                tiles[ss][:])
                        pending_o = None
                o_tiles = []
                for ss in range(4):
                    o = osb.tile([128, CW], f32, tag="o", name=f"o{q}_{ss}")
                    nc.vector.tensor_copy(o[:], o_ps[ss][:])
                    o_tiles.append(o)
                pending_o = (q, o_tiles)
            qq, tiles = pending_o
            for ss in range(4):
                nc.sync.dma_start(
                    out[qq * SQT + ss * 128:qq * SQT + (ss + 1) * 128, :],
                    tiles[ss][:])

    nc.compile()
    return nc


def _prep_inputs(x, wq, wk, wv, wo, freqs_cos, freqs_sin, mask):
    bf16 = ml_dtypes.bfloat16
    x2 = np.asarray(x, dtype=np.float32).reshape(S, D)
    xT = np.ascontiguousarray(x2.T).astype(bf16)
    cosT = np.repeat(np.asarray(freqs_cos, np.float32).T, 2, axis=0)
    sinT = np.repeat(np.asarray(freqs_sin, np.float32).T, 2, axis=0).copy()
    sinT[0::2] *= -1.0
    cosT = np.ascontiguousarray(cosT).astype(bf16)
    sinT = np.ascontiguousarray(sinT).astype(bf16)
    m2 = np.asarray(mask, np.float32).reshape(S, S)
    masks = np.stack([np.ascontiguousarray(m2[0:SQT, r * 128:(r + 1) * 128].T)
                      for r in range(4)]).astype(bf16)  # [4, 128, 512]

    in_maps = []
    for c in range(N_CORES):
        cols = slice(c * CW, (c + 1) * CW)
        in_maps.append({
            "xT": xT,
            "wq": np.ascontiguousarray(np.asarray(wq, np.float32)[:, cols]).astype(bf16),
            "wk": np.ascontiguousarray(np.asarray(wk, np.float32)[:, cols]).astype(bf16),
            "wv": np.ascontiguousarray(np.asarray(wv, np.float32)[:, cols]).astype(bf16),
            "wo": np.ascontiguousarray(np.asarray(wo, np.float32)[:, cols]).astype(bf16),
            "cosT": cosT,
            "ones": np.ones((HD, 1), bf16),
            "onesb": np.ones((128, 128), bf16),
            "sinT": sinT,
            "masks": masks,
        })
    return in_maps


def kernel(x, wq, wk, wv, wo, freqs_cos, freqs_sin, mask):
    global LAST_RESULT
    from concourse.bass_utils import run_bass_kernel_spmd

    if "nc" not in _CACHE:
        _CACHE["nc"] = _build()
    nc = _CACHE["nc"]
    in_maps = _prep_inputs(x, wq, wk, wv, wo, freqs_cos, freqs_sin, mask)
    res = run_bass_kernel_spmd(nc, in_maps, core_ids=list(range(N_CORES)))
    LAST_RESULT = res
    out = np.concatenate([res.results[c]["out"] for c in range(N_CORES)],
                         axis=1)
    return out.reshape(B, S, D).astype(np.float32)
